# revision 16
# baseline (speedup 1.0000x reference)
"""CharBiLSTM Trainium2 kernel.

Strategy:
- Embedding lookup + input projection folded into G = emb @ W_ih.T + b (host),
  so per-step input contribution is a one-hot matmul (one-hot built on host).
- Words sorted by length into 256-wide bins (equal structure across the 8
  cores -> one SPMD program); each bin runs only max-len-in-bin steps and
  finished words' hidden states are emitted at their own final step.
- Feature-major LSTM state ([hid, words]) so the recurrence needs zero
  transposes; gates evacuated from PSUM by ScalarE with fused sigmoid/tanh.
- fp32r matmuls (full PE rate at N>=256, ~1e-4 relative error).
"""

import numpy as np

N_WORDS, MAX_LEN = 16384, 16
VOCAB, EMB, HID = 128, 64, 256
NCORES = 8
BIN_W = 256  # words per bin (free dim of all matmuls); PSUM-bank limited

_LAST_RESULT = {}  # test introspection: exec_time_ns etc.


def _build_schedule(lengths):
    """Sort words by length; build per-core column schedule.

    Returns:
      core_words: [NCORES][Q] word ids (-1 = dummy), identical len-structure
      col_lens:   [Q] length of each column (same for every core)
      bins:       list of (start_col, W, S) with W == BIN_W
    """
    lengths = np.asarray(lengths)
    per_core = [[] for _ in range(NCORES)]
    col_lens = []
    # descending length order: the ragged tail bin (dummy-padded) then has
    # S=1 instead of S=16
    for L in range(MAX_LEN, 0, -1):
        idx = np.where(lengths == L)[0]
        q = -(-len(idx) // NCORES)  # ceil
        pad = q * NCORES - len(idx)
        if pad:
            idx = np.concatenate([idx, np.full(pad, -1, np.int64)])
        for c in range(NCORES):
            per_core[c].extend(idx[c * q:(c + 1) * q].tolist())
        col_lens.extend([L] * q)
    Q = len(col_lens)
    nbins = -(-Q // BIN_W)
    tot = nbins * BIN_W
    for c in range(NCORES):
        per_core[c].extend([-1] * (tot - Q))
    # dummy cols get length 0 (never emitted, never one-hot)
    col_lens = col_lens + [0] * (tot - Q)
    col_lens = np.array(col_lens, np.int64)
    bins = []
    for b in range(nbins):
        sl = col_lens[b * BIN_W:(b + 1) * BIN_W]
        S = int(sl.max())
        bins.append((b * BIN_W, BIN_W, S))
    return [np.array(w, np.int64) for w in per_core], col_lens, bins


def _emit_ranges(col_lens, start, W, S):
    """For each step t (0-based), the [a,b) column range with len == t+1."""
    sl = col_lens[start:start + W]
    out = {}
    for t in range(S):
        cols = np.where(sl == t + 1)[0]
        if len(cols):
            a, b = int(cols[0]), int(cols[-1]) + 1
            assert b - a == len(cols), "columns of equal length must be contiguous"
            out[t] = (a, b)
    return out


def _build_bass(bins, emits, TOT, OUTCOLS):
    import os
    import concourse.bacc as bacc
    import concourse.tile as tile
    from concourse import mybir

    POOL_T1 = os.environ.get("K_POOL_T1", "0") == "1"
    POOL_T2 = os.environ.get("K_POOL_T2", "0") == "1"
    BUFS = int(os.environ.get("K_BUFS", "2"))
    MM_FIRST = os.environ.get("K_MM_FIRST", "0") == "1"
    SPLIT_ACT = os.environ.get("K_SPLIT_ACT", "0") == "1"

    f32 = mybir.dt.float32
    f32r = mybir.dt.float32r
    Sig = mybir.ActivationFunctionType.Sigmoid
    Tanh = mybir.ActivationFunctionType.Tanh

    nc = bacc.Bacc(None, target_bir_lowering=False)
    d_oneh = nc.dram_tensor("oneh", [128, TOT], f32r, kind="ExternalInput")
    d_whh = nc.dram_tensor("whh", [128, 2 * 2 * 8 * 128], f32r, kind="ExternalInput")
    d_gt = nc.dram_tensor("gt", [128, 2 * 8 * 128], f32r, kind="ExternalInput")
    d_out = nc.dram_tensor("out", [128, OUTCOLS], f32, kind="ExternalOutput")

    whh_v = d_whh[:, :].rearrange("p (d k m c) -> p d k m c", d=2, k=2, m=8)
    gt_v = d_gt[:, :].rearrange("p (d m c) -> p d m c", d=2, m=8)

    with tile.TileContext(nc) as tc:
        with tc.tile_pool(name="wpool", bufs=1) as wpool, \
             tc.tile_pool(name="ohp", bufs=4) as ohp, \
             tc.tile_pool(name="psp", bufs=1, space="PSUM") as psp, \
             tc.tile_pool(name="actp", bufs=BUFS) as actp, \
             tc.tile_pool(name="stp", bufs=BUFS) as stp, \
             tc.tile_pool(name="tmpp", bufs=BUFS) as tmpp:

            whh_sb = wpool.tile([128, 2, 2, 8, 128], f32r)
            nc.sync.dma_start(out=whh_sb, in_=whh_v)
            gt_sb = wpool.tile([128, 2, 8, 128], f32r)
            nc.sync.dma_start(out=gt_sb, in_=gt_v)

            # bin start offsets in the oneh / out blobs
            oh_offs, out_offs = [], []
            acc_oh, acc_out = 0, 0
            for (start, W, S) in bins:
                oh_offs.append(acc_oh)
                out_offs.append(acc_out)
                acc_oh += 2 * W * S
                acc_out += 4 * W

            # group bins (similar step counts adjacent) -> 2*G concurrent
            # recurrence chains sharing the two PSUM slot sets
            G = int(os.environ.get("K_GROUP", "2"))
            order = sorted(range(len(bins)), key=lambda b: -bins[b][2])
            pairs = [order[i:i + G] for i in range(0, len(order), G)]

            for pair in pairs:
                state = {bi: ([None, None], [None, None]) for bi in pair}
                maxS = max(bins[bi][2] for bi in pair)
                for t in range(maxS):
                    active = [bi for bi in pair if t < bins[bi][2]]
                    ohs = {}
                    for ci, bi in enumerate(active):
                        _, W, S = bins[bi]
                        oh = ohp.tile([128, 2, W], f32r, tag="oh", name="oh")
                        nc.sync.dma_start(
                            out=oh,
                            in_=d_oneh[:, oh_offs[bi] + t * 2 * W:
                                       oh_offs[bi] + (t + 1) * 2 * W].rearrange(
                                "p (d w) -> p d w", d=2))
                        ohs[bi] = oh
                    for ci, bi in enumerate(active):
                        _emit_step(nc, tc, bins[bi], emits[bi], t, ci,
                                   state[bi], ohs[bi], whh_sb, gt_sb, d_out,
                                   out_offs[bi], actp, stp, tmpp, psp,
                                   f32, f32r, Sig, Tanh,
                                   POOL_T1, POOL_T2, SPLIT_ACT)
    nc.compile()
    return nc


def _emit_step(nc, tc, bin_, er, t, ci, state, oh, whh_sb, gt_sb, d_out,
               out_off, actp, stp, tmpp, psp, f32, f32r, Sig, Tanh,
               POOL_T1, POOL_T2, SPLIT_ACT):
    start, W, S = bin_
    h, c = state

    def emit_mms(d, ps_s, ps_g):
        # order: i,f m-tiles (0-3), then g (6,7), then o (4,5)
        # so the i/f sigmoid evacuation can start earliest
        for m in (0, 1, 2, 3, 6, 7, 4, 5):
            o_ap = ps_s[:, m, :] if m < 6 else ps_g[:, m - 6, :]
            nc.tensor.matmul(o_ap, gt_sb[:, d, m, :], oh[:, d, :],
                             start=True, stop=(t == 0))
            if t > 0:
                nc.tensor.matmul(o_ap, whh_sb[:, d, 0, m, :], h[d][:, 0, :],
                                 start=False, stop=False)
                nc.tensor.matmul(o_ap, whh_sb[:, d, 1, m, :], h[d][:, 1, :],
                                 start=False, stop=True)

    def emit_rest(d, ps_s, ps_g):
        sig = actp.tile([128, 6, W], f32, tag=f"sig{d}{ci}", name=f"sig{d}{ci}")
        tg = actp.tile([128, 2, W], f32, tag=f"tg{d}{ci}", name=f"tg{d}{ci}")
        if SPLIT_ACT:
            nc.scalar.activation(sig[:, 0:4, :], ps_s[:, 0:4, :], Sig)
            nc.scalar.activation(tg, ps_g, Tanh)
            nc.scalar.activation(sig[:, 4:6, :], ps_s[:, 4:6, :], Sig)
        else:
            nc.scalar.activation(sig, ps_s, Sig)
            nc.scalar.activation(tg, ps_g, Tanh)
        c_new = stp.tile([128, 2, W], f32, tag=f"c{d}{ci}", name=f"c{d}{ci}")
        if t == 0:
            nc.vector.tensor_mul(c_new, sig[:, 0:2, :], tg)
        else:
            t1 = tmpp.tile([128, 2, W], f32, tag=f"t1{d}{ci}", name=f"t1{d}{ci}")
            eng1 = nc.gpsimd if POOL_T1 else nc.vector
            eng1.tensor_mul(t1, sig[:, 2:4, :], c[d])
            t2 = tmpp.tile([128, 2, W], f32, tag=f"t2{d}{ci}", name=f"t2{d}{ci}")
            eng2 = nc.gpsimd if POOL_T2 else nc.vector
            eng2.tensor_mul(t2, sig[:, 0:2, :], tg)
            nc.vector.tensor_add(c_new, t1, t2)
        tc_t = tmpp.tile([128, 2, W], f32, tag=f"tc{d}{ci}", name=f"tc{d}{ci}")
        nc.scalar.activation(tc_t, c_new, Tanh)
        h_new = stp.tile([128, 2, W], f32r, tag=f"h{d}{ci}", name=f"h{d}{ci}")
        nc.vector.tensor_mul(h_new, sig[:, 4:6, :], tc_t)
        h[d], c[d] = h_new, c_new
        if t in er:
            a, b = er[t]
            dst = d_out[:, out_off + d * 2 * W:
                        out_off + (d + 1) * 2 * W].rearrange(
                "p (hh w) -> p hh w", hh=2)[:, :, a:b]
            nc.sync.dma_start(out=dst, in_=h_new[:, :, a:b].bitcast(f32))

    ps = {}
    for d in (0, 1):
        ps[d] = (psp.tile([128, 6, W], f32, tag=f"pss{d}", name=f"pss{d}"),
                 psp.tile([128, 2, W], f32, tag=f"psg{d}", name=f"psg{d}"))
    for d in (0, 1):
        emit_mms(d, *ps[d])
        emit_rest(d, *ps[d])


def _make_runner(nc, n_cores):
    """Build a reusable jitted SPMD executor for a compiled Bass module.

    Mirrors concourse.bass2jax.run_bass_via_pjrt's shard_map path, but
    keeps the jitted function so repeat calls (for timing) reuse the
    compiled NEFF instead of recompiling.
    """
    import jax
    from jax.sharding import Mesh, PartitionSpec
    from jax.experimental.shard_map import shard_map
    from concourse import bass2jax, mybir

    bass2jax.install_neuronx_cc_hook()
    assert nc.dbg_addr is None
    part_name = nc.partition_id_tensor.name if nc.partition_id_tensor else None

    in_names, out_names, out_avals, zero_outs = [], [], [], []
    for alloc in nc.m.functions[0].allocations:
        if not isinstance(alloc, mybir.MemoryLocationSet):
            continue
        name = alloc.memorylocations[0].name
        if alloc.kind == "ExternalInput":
            if name != part_name:
                in_names.append(name)
        elif alloc.kind == "ExternalOutput":
            np_dt = mybir.dt.np(alloc.dtype)
            shape = tuple(alloc.tensor_shape)
            out_avals.append(jax.core.ShapedArray(shape, np_dt))
            out_names.append(name)
            zero_outs.append(np.zeros(shape, np_dt))
    n_params = len(in_names)
    all_names = in_names + out_names
    if part_name is not None:
        all_names = all_names + [part_name]

    def _body(*args):
        operands = list(args)
        if part_name is not None:
            operands.append(bass2jax.partition_id_tensor())
        outs = bass2jax._bass_exec_p.bind(
            *operands,
            out_avals=tuple(out_avals),
            in_names=tuple(all_names),
            out_names=tuple(out_names),
            lowering_input_output_aliases=(),
            sim_require_finite=True,
            sim_require_nnan=True,
            nc=nc,
        )
        return tuple(outs)

    devices = jax.devices()[:n_cores]
    mesh = Mesh(np.asarray(devices), ("core",))
    nin = n_params + len(zero_outs)
    sharded = jax.jit(
        shard_map(_body, mesh=mesh,
                  in_specs=(PartitionSpec("core"),) * nin,
                  out_specs=(PartitionSpec("core"),) * len(out_names),
                  check_rep=False),
        keep_unused=True,
    )
    return sharded, in_names, out_names, out_avals, zero_outs


def _run_spmd(nc, in_maps, time_iters=0):
    """Execute once (returns per-core result dicts); optionally time."""
    import time as _time
    import jax

    n_cores = len(in_maps)
    sharded, in_names, out_names, out_avals, zero_outs = _make_runner(nc, n_cores)
    concat_in = [
        np.concatenate([np.asarray(in_maps[c][nm]) for c in range(n_cores)], axis=0)
        for nm in in_names
    ]
    concat_zeros = [
        np.zeros((n_cores * z.shape[0], *z.shape[1:]), z.dtype) for z in zero_outs
    ]
    dev_args = [jax.device_put(a) for a in concat_in + concat_zeros]
    out_arrs = sharded(*dev_args)
    jax.block_until_ready(out_arrs)

    exec_ns = None
    if time_iters:
        # warm
        jax.block_until_ready(sharded(*dev_args))
        t0 = _time.perf_counter()
        last = None
        for _ in range(time_iters):
            last = sharded(*dev_args)
        jax.block_until_ready(last)
        exec_ns = (_time.perf_counter() - t0) / time_iters * 1e9

    results = [
        {nm: np.asarray(out_arrs[i]).reshape(n_cores, *out_avals[i].shape)[c]
         for i, nm in enumerate(out_names)}
        for c in range(n_cores)
    ]
    return results, exec_ns


def kernel(char_ids, lengths, emb, W_ih_f, W_hh_f, b_ih_f, b_hh_f,
           W_ih_b, W_hh_b, b_ih_b, b_hh_b):
    char_ids = np.asarray(char_ids)
    lengths = np.asarray(lengths)

    # ---- host precompute: fold emb + input proj + biases into G [VOCAB, 4H]
    # permute gate order (i,f,g,o) -> (i,f,o,g) so ACT can evacuate
    # sigmoid-gates [i,f,o] with one instruction
    perm = np.concatenate([np.arange(0, 512),            # i, f
                           np.arange(768, 1024),         # o
                           np.arange(512, 768)])         # g
    outs = {}
    for d, (W_ih, W_hh, b_ih, b_hh) in enumerate(
            [(W_ih_f, W_hh_f, b_ih_f, b_hh_f),
             (W_ih_b, W_hh_b, b_ih_b, b_hh_b)]):
        G = (np.asarray(emb, np.float64) @ np.asarray(W_ih, np.float64).T
             + np.asarray(b_ih, np.float64) + np.asarray(b_hh, np.float64))
        outs[f"G{d}"] = np.ascontiguousarray(G[:, perm]).astype(np.float32)
        Wp = np.asarray(W_hh, np.float64)[perm, :].T  # [HID, 4H]
        outs[f"Wp{d}"] = Wp.astype(np.float32)

    # gt blob: [128, 2, 8, 128]
    gt = np.zeros((128, 2, 8, 128), np.float32)
    for d in range(2):
        for m in range(8):
            gt[:, d, m, :] = outs[f"G{d}"][:, m * 128:(m + 1) * 128]
    # whh blob: [128, 2, 2, 8, 128]
    whh = np.zeros((128, 2, 2, 8, 128), np.float32)
    for d in range(2):
        for k in range(2):
            for m in range(8):
                whh[:, d, k, m, :] = outs[f"Wp{d}"][
                    k * 128:(k + 1) * 128, m * 128:(m + 1) * 128]
    gt = gt.reshape(128, -1)
    whh = whh.reshape(128, -1)

    # ---- schedule
    core_words, col_lens, bins = _build_schedule(lengths)
    emits = [_emit_ranges(col_lens, s, W, S) for (s, W, S) in bins]
    TOT = sum(2 * W * S for (_, W, S) in bins)
    OUTCOLS = sum(4 * W for (_, W, _) in bins)

    # ---- one-hot blobs per core
    in_maps = []
    for cidx in range(NCORES):
        words = core_words[cidx]
        oh = np.zeros((128, TOT), np.float32)
        off = 0
        for (start, W, S) in bins:
            w_ids = words[start:start + W]
            lens = col_lens[start:start + W]
            cols = np.arange(W)
            real = w_ids >= 0
            for t in range(S):
                valid = real & (t < lens)
                if valid.any():
                    wv = w_ids[valid]
                    # fwd: char at position t
                    rows_f = char_ids[wv, t]
                    oh[rows_f, off + cols[valid]] = 1.0
                    # bwd: char at position len-1-t
                    rows_b = char_ids[wv, lens[valid] - 1 - t]
                    oh[rows_b, off + W + cols[valid]] = 1.0
                off += 2 * W
        in_maps.append({"oneh": oh, "whh": whh, "gt": gt})

    # ---- build + run
    import os
    nc = _build_bass(bins, emits, TOT, OUTCOLS)
    iters = int(os.environ.get("KERNEL_TIME_ITERS", "0"))
    results, exec_ns = _run_spmd(nc, in_maps, time_iters=iters)
    _LAST_RESULT.clear()
    _LAST_RESULT["exec_time_ns"] = exec_ns

    # ---- assemble output
    final = np.zeros((N_WORDS, 2 * HID), np.float32)
    for cidx in range(NCORES):
        out = results[cidx]["out"]
        words = core_words[cidx]
        ob = 0
        for (start, W, S) in bins:
            w_ids = words[start:start + W]
            real = w_ids >= 0
            for d in range(2):
                block = out[:, ob + d * 2 * W: ob + (d + 1) * 2 * W]
                hv = block.reshape(128, 2, W).transpose(2, 1, 0).reshape(W, 256)
                final[w_ids[real], d * HID:(d + 1) * HID] = hv[real]
            ob += 4 * W
    return final


# revision 23
# speedup vs baseline: 102.7261x; 102.7261x over previous
"""CharBiLSTM Trainium2 kernel.

Strategy:
- Embedding lookup + input projection folded into G = emb @ W_ih.T + b (host),
  so per-step input contribution is a one-hot matmul (one-hot built on host).
- Words sorted by length into 256-wide bins (equal structure across the 8
  cores -> one SPMD program); each bin runs only max-len-in-bin steps and
  finished words' hidden states are emitted at their own final step.
- Feature-major LSTM state ([hid, words]) so the recurrence needs zero
  transposes; gates evacuated from PSUM by ScalarE with fused sigmoid/tanh.
- fp32r matmuls (full PE rate at N>=256, ~1e-4 relative error).
"""

import os

import numpy as np

N_WORDS, MAX_LEN = 16384, 16
VOCAB, EMB, HID = 128, 64, 256
NCORES = 8
BIN_W = 256  # words per bin (free dim of all matmuls); PSUM-bank limited

_LAST_RESULT = {}  # test introspection: exec_time_ns etc.


def _build_schedule(lengths):
    """Sort words by length; build per-core column schedule.

    Returns:
      core_words: [NCORES][Q] word ids (-1 = dummy), identical len-structure
      col_lens:   [Q] length of each column (same for every core)
      bins:       list of (start_col, W, S) with W == BIN_W
    """
    lengths = np.asarray(lengths)
    per_core = [[] for _ in range(NCORES)]
    col_lens = []
    # descending length order: the ragged tail bin (dummy-padded) then has
    # S=1 instead of S=16
    for L in range(MAX_LEN, 0, -1):
        idx = np.where(lengths == L)[0]
        q = -(-len(idx) // NCORES)  # ceil
        pad = q * NCORES - len(idx)
        if pad:
            idx = np.concatenate([idx, np.full(pad, -1, np.int64)])
        for c in range(NCORES):
            per_core[c].extend(idx[c * q:(c + 1) * q].tolist())
        col_lens.extend([L] * q)
    Q = len(col_lens)
    nbins = -(-Q // BIN_W)
    tot = nbins * BIN_W
    for c in range(NCORES):
        per_core[c].extend([-1] * (tot - Q))
    # dummy cols get length 0 (never emitted, never one-hot)
    col_lens = col_lens + [0] * (tot - Q)
    col_lens = np.array(col_lens, np.int64)
    bins = []
    for b in range(nbins):
        sl = col_lens[b * BIN_W:(b + 1) * BIN_W]
        S = int(sl.max())
        bins.append((b * BIN_W, BIN_W, S))
    return [np.array(w, np.int64) for w in per_core], col_lens, bins


def _emit_ranges(col_lens, start, W, S):
    """For each step t (0-based), the [a,b) column range with len == t+1."""
    sl = col_lens[start:start + W]
    out = {}
    for t in range(S):
        cols = np.where(sl == t + 1)[0]
        if len(cols):
            a, b = int(cols[0]), int(cols[-1]) + 1
            assert b - a == len(cols), "columns of equal length must be contiguous"
            out[t] = (a, b)
    return out


def _build_bass(bins, emits, TOT, OUTCOLS):
    import os
    import concourse.bacc as bacc
    import concourse.tile as tile
    from concourse import mybir

    POOL_T1 = os.environ.get("K_POOL_T1", "0") == "1"
    POOL_T2 = os.environ.get("K_POOL_T2", "0") == "1"
    BUFS = int(os.environ.get("K_BUFS", "2"))
    SPLIT_ACT = os.environ.get("K_SPLIT_ACT", "0") == "1"
    SIG2 = os.environ.get("K_SIG2", "0") == "1"

    f32 = mybir.dt.float32
    f32r = mybir.dt.float32r
    Sig = mybir.ActivationFunctionType.Sigmoid
    Tanh = mybir.ActivationFunctionType.Tanh

    nc = bacc.Bacc(None, target_bir_lowering=False)
    d_oneh = nc.dram_tensor("oneh", [128, TOT], f32r, kind="ExternalInput")
    d_whh = nc.dram_tensor("whh", [128, 2 * 2 * 8 * 128], f32r, kind="ExternalInput")
    d_gt = nc.dram_tensor("gt", [128, 2 * 8 * 128], f32r, kind="ExternalInput")
    d_out = nc.dram_tensor("out", [128, OUTCOLS], f32, kind="ExternalOutput")

    whh_v = d_whh[:, :].rearrange("p (d k m c) -> p d k m c", d=2, k=2, m=8)
    gt_v = d_gt[:, :].rearrange("p (d m c) -> p d m c", d=2, m=8)

    with tile.TileContext(nc) as tc:
        with tc.tile_pool(name="wpool", bufs=1) as wpool, \
             tc.tile_pool(name="ohp", bufs=4) as ohp, \
             tc.tile_pool(name="psp", bufs=1, space="PSUM") as psp, \
             tc.tile_pool(name="actp", bufs=BUFS) as actp, \
             tc.tile_pool(name="stp", bufs=BUFS) as stp, \
             tc.tile_pool(name="tmpp", bufs=BUFS) as tmpp:

            whh_sb = wpool.tile([128, 2, 2, 8, 128], f32r)
            nc.sync.dma_start(out=whh_sb, in_=whh_v)
            gt_sb = wpool.tile([128, 2, 8, 128], f32r)
            nc.sync.dma_start(out=gt_sb, in_=gt_v)

            # bin start offsets in the oneh / out blobs
            oh_offs, out_offs = [], []
            acc_oh, acc_out = 0, 0
            for (start, W, S) in bins:
                oh_offs.append(acc_oh)
                out_offs.append(acc_out)
                acc_oh += 2 * W * S
                acc_out += 4 * W

            # group bins (similar step counts adjacent) -> 2*G concurrent
            # recurrence chains sharing the two PSUM slot sets
            G = int(os.environ.get("K_GROUP", "2"))
            order = sorted(range(len(bins)), key=lambda b: -bins[b][2])
            pairs = [order[i:i + G] for i in range(0, len(order), G)]

            for pair in pairs:
                state = {bi: ([None, None], [None, None]) for bi in pair}
                maxS = max(bins[bi][2] for bi in pair)
                for t in range(maxS):
                    active = [bi for bi in pair if t < bins[bi][2]]
                    ohs = {}
                    for ci, bi in enumerate(active):
                        _, W, S = bins[bi]
                        oh = ohp.tile([128, 2, W], f32r, tag="oh", name="oh")
                        nc.sync.dma_start(
                            out=oh,
                            in_=d_oneh[:, oh_offs[bi] + t * 2 * W:
                                       oh_offs[bi] + (t + 1) * 2 * W].rearrange(
                                "p (d w) -> p d w", d=2))
                        ohs[bi] = oh
                    for ci, bi in enumerate(active):
                        if SIG2:
                            _emit_step_sig2(nc, bins[bi], emits[bi], t, ci,
                                            state[bi], ohs[bi], whh_sb, gt_sb,
                                            d_out, out_offs[bi], actp, stp,
                                            tmpp, psp, f32, f32r, Sig, Tanh,
                                            POOL_T1, mybir)
                        else:
                            _emit_step(nc, tc, bins[bi], emits[bi], t, ci,
                                       state[bi], ohs[bi], whh_sb, gt_sb, d_out,
                                       out_offs[bi], actp, stp, tmpp, psp,
                                       f32, f32r, Sig, Tanh,
                                       POOL_T1, POOL_T2, SPLIT_ACT)
    nc.compile()
    return nc


def _emit_step(nc, tc, bin_, er, t, ci, state, oh, whh_sb, gt_sb, d_out,
               out_off, actp, stp, tmpp, psp, f32, f32r, Sig, Tanh,
               POOL_T1, POOL_T2, SPLIT_ACT):
    start, W, S = bin_
    h, c = state

    def emit_mms(d, ps_s, ps_g):
        # order: i,f m-tiles (0-3), then g (6,7), then o (4,5)
        # so the i/f sigmoid evacuation can start earliest
        for m in (0, 1, 2, 3, 6, 7, 4, 5):
            o_ap = ps_s[:, m, :] if m < 6 else ps_g[:, m - 6, :]
            nc.tensor.matmul(o_ap, gt_sb[:, d, m, :], oh[:, d, :],
                             start=True, stop=(t == 0))
            if t > 0:
                nc.tensor.matmul(o_ap, whh_sb[:, d, 0, m, :], h[d][:, 0, :],
                                 start=False, stop=False)
                nc.tensor.matmul(o_ap, whh_sb[:, d, 1, m, :], h[d][:, 1, :],
                                 start=False, stop=True)

    def emit_rest(d, ps_s, ps_g):
        sig = actp.tile([128, 6, W], f32, tag=f"sig{d}{ci}", name=f"sig{d}{ci}")
        tg = actp.tile([128, 2, W], f32, tag=f"tg{d}{ci}", name=f"tg{d}{ci}")
        if SPLIT_ACT:
            nc.scalar.activation(sig[:, 0:4, :], ps_s[:, 0:4, :], Sig)
            nc.scalar.activation(tg, ps_g, Tanh)
            nc.scalar.activation(sig[:, 4:6, :], ps_s[:, 4:6, :], Sig)
        else:
            nc.scalar.activation(sig, ps_s, Sig)
            nc.scalar.activation(tg, ps_g, Tanh)
        c_new = stp.tile([128, 2, W], f32, tag=f"c{d}{ci}", name=f"c{d}{ci}")
        if t == 0:
            nc.vector.tensor_mul(c_new, sig[:, 0:2, :], tg)
        else:
            t1 = tmpp.tile([128, 2, W], f32, tag=f"t1{d}{ci}", name=f"t1{d}{ci}")
            eng1 = nc.gpsimd if POOL_T1 else nc.vector
            eng1.tensor_mul(t1, sig[:, 2:4, :], c[d])
            t2 = tmpp.tile([128, 2, W], f32, tag=f"t2{d}{ci}", name=f"t2{d}{ci}")
            eng2 = nc.gpsimd if POOL_T2 else nc.vector
            eng2.tensor_mul(t2, sig[:, 0:2, :], tg)
            nc.vector.tensor_add(c_new, t1, t2)
        tc_t = tmpp.tile([128, 2, W], f32, tag=f"tc{d}{ci}", name=f"tc{d}{ci}")
        nc.scalar.activation(tc_t, c_new, Tanh)
        h_new = stp.tile([128, 2, W], f32r, tag=f"h{d}{ci}", name=f"h{d}{ci}")
        nc.vector.tensor_mul(h_new, sig[:, 4:6, :], tc_t)
        h[d], c[d] = h_new, c_new
        if t in er:
            a, b = er[t]
            dst = d_out[:, out_off + d * 2 * W:
                        out_off + (d + 1) * 2 * W].rearrange(
                "p (hh w) -> p hh w", hh=2)[:, :, a:b]
            nc.sync.dma_start(out=dst, in_=h_new[:, :, a:b].bitcast(f32))

    ps = {}
    for d in (0, 1):
        ps[d] = (psp.tile([128, 6, W], f32, tag=f"pss{d}", name=f"pss{d}"),
                 psp.tile([128, 2, W], f32, tag=f"psg{d}", name=f"psg{d}"))
    for d in (0, 1):
        emit_mms(d, *ps[d])
        emit_rest(d, *ps[d])


def _emit_step_sig2(nc, bin_, er, t, ci, state, oh, whh_sb, gt_sb, d_out,
                    out_off, actp, stp, tmpp, psp, f32, f32r, Sig, Tanh,
                    POOL_T1, mybir):
    """Variant: g-gate pre-scaled by 2 on host so tanh(g) = 2*sigmoid(2g)-1;
    all four gates evacuate through ONE sigmoid ACT instruction."""
    start, W, S = bin_
    h, c = state
    mult = mybir.AluOpType.mult

    for d in (0, 1):
        ps = psp.tile([128, 8, W], f32, tag=f"ps{d}", name=f"ps{d}")
        for m in (0, 1, 2, 3, 6, 7, 4, 5):
            o_ap = ps[:, m, :]
            nc.tensor.matmul(o_ap, gt_sb[:, d, m, :], oh[:, d, :],
                             start=True, stop=(t == 0))
            if t > 0:
                nc.tensor.matmul(o_ap, whh_sb[:, d, 0, m, :], h[d][:, 0, :],
                                 start=False, stop=False)
                nc.tensor.matmul(o_ap, whh_sb[:, d, 1, m, :], h[d][:, 1, :],
                                 start=False, stop=True)
        sg = actp.tile([128, 8, W], f32, tag=f"sg{d}{ci}", name=f"sg{d}{ci}")
        nc.scalar.activation(sg, ps, Sig)
        si, sf, so, gg = (sg[:, 0:2, :], sg[:, 2:4, :],
                          sg[:, 4:6, :], sg[:, 6:8, :])
        c_new = stp.tile([128, 2, W], f32, tag=f"c{d}{ci}", name=f"c{d}{ci}")
        t2 = tmpp.tile([128, 2, W], f32, tag=f"t2{d}{ci}", name=f"t2{d}{ci}")
        # t2 = (2*sigmoid(2g)) * sigmoid(i)
        nc.vector.scalar_tensor_tensor(t2, gg, 2.0, si, op0=mult, op1=mult)
        if t == 0:
            nc.vector.tensor_sub(c_new, t2, si)  # c = i*(2sg-1)
        else:
            t1 = tmpp.tile([128, 2, W], f32, tag=f"t1{d}{ci}", name=f"t1{d}{ci}")
            eng1 = nc.gpsimd if POOL_T1 else nc.vector
            eng1.tensor_mul(t1, sf, c[d])
            t3 = tmpp.tile([128, 2, W], f32, tag=f"t3{d}{ci}", name=f"t3{d}{ci}")
            nc.vector.tensor_sub(t3, t1, si)
            nc.vector.tensor_add(c_new, t3, t2)
        tc_t = tmpp.tile([128, 2, W], f32, tag=f"tc{d}{ci}", name=f"tc{d}{ci}")
        nc.scalar.activation(tc_t, c_new, Tanh)
        h_new = stp.tile([128, 2, W], f32r, tag=f"h{d}{ci}", name=f"h{d}{ci}")
        nc.vector.tensor_mul(h_new, so, tc_t)
        h[d], c[d] = h_new, c_new
        if t in er:
            a, b = er[t]
            dst = d_out[:, out_off + d * 2 * W:
                        out_off + (d + 1) * 2 * W].rearrange(
                "p (hh w) -> p hh w", hh=2)[:, :, a:b]
            nc.sync.dma_start(out=dst, in_=h_new[:, :, a:b].bitcast(f32))


def _make_runner(nc, n_cores):
    """Build a reusable jitted SPMD executor for a compiled Bass module.

    Mirrors concourse.bass2jax.run_bass_via_pjrt's shard_map path, but
    keeps the jitted function so repeat calls (for timing) reuse the
    compiled NEFF instead of recompiling.
    """
    import jax
    from jax.sharding import Mesh, PartitionSpec
    from jax.experimental.shard_map import shard_map
    from concourse import bass2jax, mybir

    bass2jax.install_neuronx_cc_hook()
    assert nc.dbg_addr is None
    part_name = nc.partition_id_tensor.name if nc.partition_id_tensor else None

    in_names, out_names, out_avals, zero_outs = [], [], [], []
    for alloc in nc.m.functions[0].allocations:
        if not isinstance(alloc, mybir.MemoryLocationSet):
            continue
        name = alloc.memorylocations[0].name
        if alloc.kind == "ExternalInput":
            if name != part_name:
                in_names.append(name)
        elif alloc.kind == "ExternalOutput":
            np_dt = mybir.dt.np(alloc.dtype)
            shape = tuple(alloc.tensor_shape)
            out_avals.append(jax.core.ShapedArray(shape, np_dt))
            out_names.append(name)
            zero_outs.append(np.zeros(shape, np_dt))
    n_params = len(in_names)
    all_names = in_names + out_names
    if part_name is not None:
        all_names = all_names + [part_name]

    def _body(*args):
        operands = list(args)
        if part_name is not None:
            operands.append(bass2jax.partition_id_tensor())
        outs = bass2jax._bass_exec_p.bind(
            *operands,
            out_avals=tuple(out_avals),
            in_names=tuple(all_names),
            out_names=tuple(out_names),
            lowering_input_output_aliases=(),
            sim_require_finite=True,
            sim_require_nnan=True,
            nc=nc,
        )
        return tuple(outs)

    devices = jax.devices()[:n_cores]
    mesh = Mesh(np.asarray(devices), ("core",))
    nin = n_params + len(zero_outs)
    sharded = jax.jit(
        shard_map(_body, mesh=mesh,
                  in_specs=(PartitionSpec("core"),) * nin,
                  out_specs=(PartitionSpec("core"),) * len(out_names),
                  check_rep=False),
        keep_unused=True,
    )
    return sharded, in_names, out_names, out_avals, zero_outs


def _run_spmd(nc, in_maps, time_iters=0):
    """Execute once (returns per-core result dicts); optionally time."""
    import time as _time
    import jax

    n_cores = len(in_maps)
    sharded, in_names, out_names, out_avals, zero_outs = _make_runner(nc, n_cores)
    concat_in = [
        np.concatenate([np.asarray(in_maps[c][nm]) for c in range(n_cores)], axis=0)
        for nm in in_names
    ]
    concat_zeros = [
        np.zeros((n_cores * z.shape[0], *z.shape[1:]), z.dtype) for z in zero_outs
    ]
    dev_args = [jax.device_put(a) for a in concat_in + concat_zeros]
    out_arrs = sharded(*dev_args)
    jax.block_until_ready(out_arrs)

    exec_ns = None
    if time_iters:
        # warm
        jax.block_until_ready(sharded(*dev_args))
        t0 = _time.perf_counter()
        last = None
        for _ in range(time_iters):
            last = sharded(*dev_args)
        jax.block_until_ready(last)
        exec_ns = (_time.perf_counter() - t0) / time_iters * 1e9

    results = [
        {nm: np.asarray(out_arrs[i]).reshape(n_cores, *out_avals[i].shape)[c]
         for i, nm in enumerate(out_names)}
        for c in range(n_cores)
    ]
    return results, exec_ns


def kernel(char_ids, lengths, emb, W_ih_f, W_hh_f, b_ih_f, b_hh_f,
           W_ih_b, W_hh_b, b_ih_b, b_hh_b):
    char_ids = np.asarray(char_ids)
    lengths = np.asarray(lengths)

    # ---- host precompute: fold emb + input proj + biases into G [VOCAB, 4H]
    # permute gate order (i,f,g,o) -> (i,f,o,g) so ACT can evacuate
    # sigmoid-gates [i,f,o] with one instruction
    perm = np.concatenate([np.arange(0, 512),            # i, f
                           np.arange(768, 1024),         # o
                           np.arange(512, 768)])         # g
    sig2 = os.environ.get("K_SIG2", "0") == "1"
    outs = {}
    for d, (W_ih, W_hh, b_ih, b_hh) in enumerate(
            [(W_ih_f, W_hh_f, b_ih_f, b_hh_f),
             (W_ih_b, W_hh_b, b_ih_b, b_hh_b)]):
        G = (np.asarray(emb, np.float64) @ np.asarray(W_ih, np.float64).T
             + np.asarray(b_ih, np.float64) + np.asarray(b_hh, np.float64))
        Gp = np.ascontiguousarray(G[:, perm])
        Wp = np.asarray(W_hh, np.float64)[perm, :].T  # [HID, 4H]
        Wp = np.ascontiguousarray(Wp)
        if sig2:
            # tanh(g) computed as 2*sigmoid(2g)-1: pre-scale g block (cols
            # 768:1024 after permutation) by 2
            Gp[:, 768:1024] *= 2.0
            Wp[:, 768:1024] *= 2.0
        outs[f"G{d}"] = Gp.astype(np.float32)
        outs[f"Wp{d}"] = Wp.astype(np.float32)

    # gt blob: [128, 2, 8, 128]
    gt = np.zeros((128, 2, 8, 128), np.float32)
    for d in range(2):
        for m in range(8):
            gt[:, d, m, :] = outs[f"G{d}"][:, m * 128:(m + 1) * 128]
    # whh blob: [128, 2, 2, 8, 128]
    whh = np.zeros((128, 2, 2, 8, 128), np.float32)
    for d in range(2):
        for k in range(2):
            for m in range(8):
                whh[:, d, k, m, :] = outs[f"Wp{d}"][
                    k * 128:(k + 1) * 128, m * 128:(m + 1) * 128]
    gt = gt.reshape(128, -1)
    whh = whh.reshape(128, -1)

    # ---- schedule
    core_words, col_lens, bins = _build_schedule(lengths)
    emits = [_emit_ranges(col_lens, s, W, S) for (s, W, S) in bins]
    TOT = sum(2 * W * S for (_, W, S) in bins)
    OUTCOLS = sum(4 * W for (_, W, _) in bins)

    # ---- one-hot blobs per core
    in_maps = []
    for cidx in range(NCORES):
        words = core_words[cidx]
        oh = np.zeros((128, TOT), np.float32)
        off = 0
        for (start, W, S) in bins:
            w_ids = words[start:start + W]
            lens = col_lens[start:start + W]
            cols = np.arange(W)
            real = w_ids >= 0
            for t in range(S):
                valid = real & (t < lens)
                if valid.any():
                    wv = w_ids[valid]
                    # fwd: char at position t
                    rows_f = char_ids[wv, t]
                    oh[rows_f, off + cols[valid]] = 1.0
                    # bwd: char at position len-1-t
                    rows_b = char_ids[wv, lens[valid] - 1 - t]
                    oh[rows_b, off + W + cols[valid]] = 1.0
                off += 2 * W
        in_maps.append({"oneh": oh, "whh": whh, "gt": gt})

    # ---- build + run
    nc = _build_bass(bins, emits, TOT, OUTCOLS)
    iters = int(os.environ.get("KERNEL_TIME_ITERS", "0"))
    results, exec_ns = _run_spmd(nc, in_maps, time_iters=iters)
    _LAST_RESULT.clear()
    _LAST_RESULT["exec_time_ns"] = exec_ns
    _LAST_RESULT["nc"] = nc

    # ---- assemble output
    final = np.zeros((N_WORDS, 2 * HID), np.float32)
    for cidx in range(NCORES):
        out = results[cidx]["out"]
        words = core_words[cidx]
        ob = 0
        for (start, W, S) in bins:
            w_ids = words[start:start + W]
            real = w_ids >= 0
            for d in range(2):
                block = out[:, ob + d * 2 * W: ob + (d + 1) * 2 * W]
                hv = block.reshape(128, 2, W).transpose(2, 1, 0).reshape(W, 256)
                final[w_ids[real], d * HID:(d + 1) * HID] = hv[real]
            ob += 4 * W
    return final


# revision 31
# speedup vs baseline: 107.4984x; 1.0465x over previous
"""CharBiLSTM Trainium2 kernel.

Strategy:
- Embedding lookup + input projection folded into G = emb @ W_ih.T + b (host),
  so per-step input contribution is a one-hot matmul (one-hot built on host).
- Words sorted by length into 256-wide bins (equal structure across the 8
  cores -> one SPMD program); each bin runs only max-len-in-bin steps and
  finished words' hidden states are emitted at their own final step.
- Feature-major LSTM state ([hid, words]) so the recurrence needs zero
  transposes; gates evacuated from PSUM by ScalarE with fused sigmoid/tanh.
- fp32r matmuls (full PE rate at N>=256, ~1e-4 relative error).
"""

import os

import numpy as np

N_WORDS, MAX_LEN = 16384, 16
VOCAB, EMB, HID = 128, 64, 256
NCORES = 8
BIN_W = 256  # words per bin (free dim of all matmuls); PSUM-bank limited

_LAST_RESULT = {}  # test introspection: exec_time_ns etc.


def _build_schedule(lengths):
    """Sort words by length; build per-core column schedule.

    Returns:
      core_words: [NCORES][Q] word ids (-1 = dummy), identical len-structure
      col_lens:   [Q] length of each column (same for every core)
      bins:       list of (start_col, W, S) with W == BIN_W
    """
    lengths = np.asarray(lengths)
    per_core = [[] for _ in range(NCORES)]
    col_lens = []
    # descending length order: the ragged tail bin (dummy-padded) then has
    # S=1 instead of S=16
    for L in range(MAX_LEN, 0, -1):
        idx = np.where(lengths == L)[0]
        q = -(-len(idx) // NCORES)  # ceil
        pad = q * NCORES - len(idx)
        if pad:
            idx = np.concatenate([idx, np.full(pad, -1, np.int64)])
        for c in range(NCORES):
            per_core[c].extend(idx[c * q:(c + 1) * q].tolist())
        col_lens.extend([L] * q)
    Q = len(col_lens)
    nbins = -(-Q // BIN_W)
    tot = nbins * BIN_W
    for c in range(NCORES):
        per_core[c].extend([-1] * (tot - Q))
    # dummy cols get length 0 (never emitted, never one-hot)
    col_lens = col_lens + [0] * (tot - Q)
    col_lens = np.array(col_lens, np.int64)
    bins = []
    for b in range(nbins):
        sl = col_lens[b * BIN_W:(b + 1) * BIN_W]
        S = int(sl.max())
        bins.append((b * BIN_W, BIN_W, S))
    return [np.array(w, np.int64) for w in per_core], col_lens, bins


def _emit_ranges(col_lens, start, W, S):
    """For each step t (0-based), the [a,b) column range with len == t+1."""
    sl = col_lens[start:start + W]
    out = {}
    for t in range(S):
        cols = np.where(sl == t + 1)[0]
        if len(cols):
            a, b = int(cols[0]), int(cols[-1]) + 1
            assert b - a == len(cols), "columns of equal length must be contiguous"
            out[t] = (a, b)
    return out


def _live_widths(col_lens, start, W, S):
    """Live column count at each step (cols with len >= t+1; descending
    order makes them a prefix). Elementwise/ACT ops are sliced to this;
    matmuls stay full width (fp32r needs N>=256)."""
    sl = col_lens[start:start + W]
    return [int(np.sum(sl >= t + 1)) for t in range(S)]


def _build_bass(bins, emits, TOT, OUTCOLS, lws=None):
    import os
    import concourse.bacc as bacc
    import concourse.tile as tile
    from concourse import mybir

    POOL_T1 = os.environ.get("K_POOL_T1", "0") == "1"
    POOL_T2 = os.environ.get("K_POOL_T2", "0") == "1"
    BUFS = int(os.environ.get("K_BUFS", "2"))
    ABUFS = int(os.environ.get("K_ABUFS", str(BUFS)))
    TBUFS = int(os.environ.get("K_TBUFS", str(BUFS)))
    SPLIT_ACT = os.environ.get("K_SPLIT_ACT", "0") == "1"
    SIG2 = os.environ.get("K_SIG2", "0") == "1"

    f32 = mybir.dt.float32
    f32r = mybir.dt.float32r
    Sig = mybir.ActivationFunctionType.Sigmoid
    Tanh = mybir.ActivationFunctionType.Tanh

    nc = bacc.Bacc(None, target_bir_lowering=False)
    d_oneh = nc.dram_tensor("oneh", [128, TOT], f32r, kind="ExternalInput")
    d_whh = nc.dram_tensor("whh", [128, 2 * 2 * 8 * 128], f32r, kind="ExternalInput")
    d_gt = nc.dram_tensor("gt", [128, 2 * 8 * 128], f32r, kind="ExternalInput")
    d_out = nc.dram_tensor("out", [128, OUTCOLS], f32, kind="ExternalOutput")

    whh_v = d_whh[:, :].rearrange("p (d k m c) -> p d k m c", d=2, k=2, m=8)
    gt_v = d_gt[:, :].rearrange("p (d m c) -> p d m c", d=2, m=8)

    with tile.TileContext(nc) as tc:
        with tc.tile_pool(name="wpool", bufs=1) as wpool, \
             tc.tile_pool(name="ohp", bufs=4) as ohp, \
             tc.tile_pool(name="psp", bufs=1, space="PSUM") as psp, \
             tc.tile_pool(name="actp", bufs=ABUFS) as actp, \
             tc.tile_pool(name="stp", bufs=BUFS) as stp, \
             tc.tile_pool(name="tmpp", bufs=TBUFS) as tmpp:

            whh_sb = wpool.tile([128, 2, 2, 8, 128], f32r)
            nc.sync.dma_start(out=whh_sb, in_=whh_v)
            gt_sb = wpool.tile([128, 2, 8, 128], f32r)
            nc.sync.dma_start(out=gt_sb, in_=gt_v)

            # bin start offsets in the oneh / out blobs
            oh_offs, out_offs = [], []
            acc_oh, acc_out = 0, 0
            for (start, W, S) in bins:
                oh_offs.append(acc_oh)
                out_offs.append(acc_out)
                acc_oh += 2 * W * S
                acc_out += 4 * W

            # group bins (similar step counts adjacent) -> 2*G concurrent
            # recurrence chains sharing the two PSUM slot sets
            G = int(os.environ.get("K_GROUP", "2"))
            order = sorted(range(len(bins)), key=lambda b: -bins[b][2])
            pairs = [order[i:i + G] for i in range(0, len(order), G)]

            for pair in pairs:
                state = {bi: ([None, None], [None, None]) for bi in pair}
                maxS = max(bins[bi][2] for bi in pair)
                for t in range(maxS):
                    active = [bi for bi in pair if t < bins[bi][2]]
                    ohs = {}
                    for ci, bi in enumerate(active):
                        _, W, S = bins[bi]
                        oh = ohp.tile([128, 2, W], f32r, tag="oh", name="oh")
                        nc.sync.dma_start(
                            out=oh,
                            in_=d_oneh[:, oh_offs[bi] + t * 2 * W:
                                       oh_offs[bi] + (t + 1) * 2 * W].rearrange(
                                "p (d w) -> p d w", d=2))
                        ohs[bi] = oh
                    for ci, bi in enumerate(active):
                        if SIG2:
                            _emit_step_sig2(nc, bins[bi], emits[bi], t, ci,
                                            state[bi], ohs[bi], whh_sb, gt_sb,
                                            d_out, out_offs[bi], actp, stp,
                                            tmpp, psp, f32, f32r, Sig, Tanh,
                                            POOL_T1, mybir)
                        else:
                            lw = lws[bi][t] if lws is not None else bins[bi][1]
                            _emit_step(nc, tc, bins[bi], emits[bi], t, ci,
                                       state[bi], ohs[bi], whh_sb, gt_sb, d_out,
                                       out_offs[bi], actp, stp, tmpp, psp,
                                       f32, f32r, Sig, Tanh,
                                       POOL_T1, POOL_T2, SPLIT_ACT, lw)
    nc.compile()
    return nc


def _emit_step(nc, tc, bin_, er, t, ci, state, oh, whh_sb, gt_sb, d_out,
               out_off, actp, stp, tmpp, psp, f32, f32r, Sig, Tanh,
               POOL_T1, POOL_T2, SPLIT_ACT, lw=None):
    start, W, S = bin_
    if lw is None or lw > W:
        lw = W
    h, c = state

    def emit_mms(d, ps_s, ps_g):
        # order: i,f m-tiles (0-3), then g (6,7), then o (4,5)
        # so the i/f sigmoid evacuation can start earliest
        for m in (0, 1, 2, 3, 6, 7, 4, 5):
            o_ap = ps_s[:, m, :] if m < 6 else ps_g[:, m - 6, :]
            nc.tensor.matmul(o_ap, gt_sb[:, d, m, :], oh[:, d, :],
                             start=True, stop=(t == 0))
            if t > 0:
                nc.tensor.matmul(o_ap, whh_sb[:, d, 0, m, :], h[d][:, 0, :],
                                 start=False, stop=False)
                nc.tensor.matmul(o_ap, whh_sb[:, d, 1, m, :], h[d][:, 1, :],
                                 start=False, stop=True)

    def emit_rest(d, ps_s, ps_g):
        sig = actp.tile([128, 6, W], f32, tag=f"sig{d}{ci}", name=f"sig{d}{ci}")
        tg = actp.tile([128, 2, W], f32, tag=f"tg{d}{ci}", name=f"tg{d}{ci}")
        if SPLIT_ACT:
            nc.scalar.activation(sig[:, 0:4, 0:lw], ps_s[:, 0:4, 0:lw], Sig)
            nc.scalar.activation(tg[:, :, 0:lw], ps_g[:, :, 0:lw], Tanh)
            nc.scalar.activation(sig[:, 4:6, 0:lw], ps_s[:, 4:6, 0:lw], Sig)
        else:
            nc.scalar.activation(sig[:, :, 0:lw], ps_s[:, :, 0:lw], Sig)
            nc.scalar.activation(tg[:, :, 0:lw], ps_g[:, :, 0:lw], Tanh)
        c_new = stp.tile([128, 2, W], f32, tag=f"c{d}{ci}", name=f"c{d}{ci}")
        if t == 0:
            nc.vector.tensor_mul(c_new[:, :, 0:lw], sig[:, 0:2, 0:lw],
                                 tg[:, :, 0:lw])
        else:
            t1 = tmpp.tile([128, 2, W], f32, tag=f"t1{d}{ci}", name=f"t1{d}{ci}")
            eng1 = nc.gpsimd if POOL_T1 else nc.vector
            eng1.tensor_mul(t1[:, :, 0:lw], sig[:, 2:4, 0:lw], c[d][:, :, 0:lw])
            t2 = tmpp.tile([128, 2, W], f32, tag=f"t2{d}{ci}", name=f"t2{d}{ci}")
            eng2 = nc.gpsimd if POOL_T2 else nc.vector
            eng2.tensor_mul(t2[:, :, 0:lw], sig[:, 0:2, 0:lw], tg[:, :, 0:lw])
            nc.vector.tensor_add(c_new[:, :, 0:lw], t1[:, :, 0:lw],
                                 t2[:, :, 0:lw])
        tc_t = tmpp.tile([128, 2, W], f32, tag=f"tc{d}{ci}", name=f"tc{d}{ci}")
        nc.scalar.activation(tc_t[:, :, 0:lw], c_new[:, :, 0:lw], Tanh)
        h_new = stp.tile([128, 2, W], f32r, tag=f"h{d}{ci}", name=f"h{d}{ci}")
        nc.vector.tensor_mul(h_new[:, :, 0:lw], sig[:, 4:6, 0:lw],
                             tc_t[:, :, 0:lw])
        h[d], c[d] = h_new, c_new
        if t in er:
            a, b = er[t]
            dst = d_out[:, out_off + d * 2 * W:
                        out_off + (d + 1) * 2 * W].rearrange(
                "p (hh w) -> p hh w", hh=2)[:, :, a:b]
            nc.sync.dma_start(out=dst, in_=h_new[:, :, a:b].bitcast(f32))

    ps = {}
    for d in (0, 1):
        ps[d] = (psp.tile([128, 6, W], f32, tag=f"pss{d}", name=f"pss{d}"),
                 psp.tile([128, 2, W], f32, tag=f"psg{d}", name=f"psg{d}"))
    for d in (0, 1):
        emit_mms(d, *ps[d])
        emit_rest(d, *ps[d])


def _emit_step_sig2(nc, bin_, er, t, ci, state, oh, whh_sb, gt_sb, d_out,
                    out_off, actp, stp, tmpp, psp, f32, f32r, Sig, Tanh,
                    POOL_T1, mybir):
    """Variant: g-gate pre-scaled by 2 on host so tanh(g) = 2*sigmoid(2g)-1;
    all four gates evacuate through ONE sigmoid ACT instruction."""
    start, W, S = bin_
    h, c = state
    mult = mybir.AluOpType.mult

    for d in (0, 1):
        ps = psp.tile([128, 8, W], f32, tag=f"ps{d}", name=f"ps{d}")
        for m in (0, 1, 2, 3, 6, 7, 4, 5):
            o_ap = ps[:, m, :]
            nc.tensor.matmul(o_ap, gt_sb[:, d, m, :], oh[:, d, :],
                             start=True, stop=(t == 0))
            if t > 0:
                nc.tensor.matmul(o_ap, whh_sb[:, d, 0, m, :], h[d][:, 0, :],
                                 start=False, stop=False)
                nc.tensor.matmul(o_ap, whh_sb[:, d, 1, m, :], h[d][:, 1, :],
                                 start=False, stop=True)
        sg = actp.tile([128, 8, W], f32, tag=f"sg{d}{ci}", name=f"sg{d}{ci}")
        nc.scalar.activation(sg, ps, Sig)
        si, sf, so, gg = (sg[:, 0:2, :], sg[:, 2:4, :],
                          sg[:, 4:6, :], sg[:, 6:8, :])
        c_new = stp.tile([128, 2, W], f32, tag=f"c{d}{ci}", name=f"c{d}{ci}")
        t2 = tmpp.tile([128, 2, W], f32, tag=f"t2{d}{ci}", name=f"t2{d}{ci}")
        # t2 = (2*sigmoid(2g)) * sigmoid(i)
        nc.vector.scalar_tensor_tensor(t2, gg, 2.0, si, op0=mult, op1=mult)
        if t == 0:
            nc.vector.tensor_sub(c_new, t2, si)  # c = i*(2sg-1)
        else:
            t1 = tmpp.tile([128, 2, W], f32, tag=f"t1{d}{ci}", name=f"t1{d}{ci}")
            eng1 = nc.gpsimd if POOL_T1 else nc.vector
            eng1.tensor_mul(t1, sf, c[d])
            t3 = tmpp.tile([128, 2, W], f32, tag=f"t3{d}{ci}", name=f"t3{d}{ci}")
            nc.vector.tensor_sub(t3, t1, si)
            nc.vector.tensor_add(c_new, t3, t2)
        tc_t = tmpp.tile([128, 2, W], f32, tag=f"tc{d}{ci}", name=f"tc{d}{ci}")
        nc.scalar.activation(tc_t, c_new, Tanh)
        h_new = stp.tile([128, 2, W], f32r, tag=f"h{d}{ci}", name=f"h{d}{ci}")
        nc.vector.tensor_mul(h_new, so, tc_t)
        h[d], c[d] = h_new, c_new
        if t in er:
            a, b = er[t]
            dst = d_out[:, out_off + d * 2 * W:
                        out_off + (d + 1) * 2 * W].rearrange(
                "p (hh w) -> p hh w", hh=2)[:, :, a:b]
            nc.sync.dma_start(out=dst, in_=h_new[:, :, a:b].bitcast(f32))


def _make_runner(nc, n_cores):
    """Build a reusable jitted SPMD executor for a compiled Bass module.

    Mirrors concourse.bass2jax.run_bass_via_pjrt's shard_map path, but
    keeps the jitted function so repeat calls (for timing) reuse the
    compiled NEFF instead of recompiling.
    """
    import jax
    from jax.sharding import Mesh, PartitionSpec
    from jax.experimental.shard_map import shard_map
    from concourse import bass2jax, mybir

    bass2jax.install_neuronx_cc_hook()
    assert nc.dbg_addr is None
    part_name = nc.partition_id_tensor.name if nc.partition_id_tensor else None

    in_names, out_names, out_avals, zero_outs = [], [], [], []
    for alloc in nc.m.functions[0].allocations:
        if not isinstance(alloc, mybir.MemoryLocationSet):
            continue
        name = alloc.memorylocations[0].name
        if alloc.kind == "ExternalInput":
            if name != part_name:
                in_names.append(name)
        elif alloc.kind == "ExternalOutput":
            np_dt = mybir.dt.np(alloc.dtype)
            shape = tuple(alloc.tensor_shape)
            out_avals.append(jax.core.ShapedArray(shape, np_dt))
            out_names.append(name)
            zero_outs.append(np.zeros(shape, np_dt))
    n_params = len(in_names)
    all_names = in_names + out_names
    if part_name is not None:
        all_names = all_names + [part_name]

    def _body(*args):
        operands = list(args)
        if part_name is not None:
            operands.append(bass2jax.partition_id_tensor())
        outs = bass2jax._bass_exec_p.bind(
            *operands,
            out_avals=tuple(out_avals),
            in_names=tuple(all_names),
            out_names=tuple(out_names),
            lowering_input_output_aliases=(),
            sim_require_finite=True,
            sim_require_nnan=True,
            nc=nc,
        )
        return tuple(outs)

    devices = jax.devices()[:n_cores]
    mesh = Mesh(np.asarray(devices), ("core",))
    nin = n_params + len(zero_outs)
    sharded = jax.jit(
        shard_map(_body, mesh=mesh,
                  in_specs=(PartitionSpec("core"),) * nin,
                  out_specs=(PartitionSpec("core"),) * len(out_names),
                  check_rep=False),
        keep_unused=True,
    )
    return sharded, in_names, out_names, out_avals, zero_outs


def _run_spmd(nc, in_maps, time_iters=0):
    """Execute once (returns per-core result dicts); optionally time."""
    import time as _time
    import jax

    n_cores = len(in_maps)
    sharded, in_names, out_names, out_avals, zero_outs = _make_runner(nc, n_cores)
    concat_in = [
        np.concatenate([np.asarray(in_maps[c][nm]) for c in range(n_cores)], axis=0)
        for nm in in_names
    ]
    concat_zeros = [
        np.zeros((n_cores * z.shape[0], *z.shape[1:]), z.dtype) for z in zero_outs
    ]
    dev_args = [jax.device_put(a) for a in concat_in + concat_zeros]
    out_arrs = sharded(*dev_args)
    jax.block_until_ready(out_arrs)

    exec_ns = None
    if time_iters:
        # warm
        jax.block_until_ready(sharded(*dev_args))
        t0 = _time.perf_counter()
        last = None
        for _ in range(time_iters):
            last = sharded(*dev_args)
        jax.block_until_ready(last)
        exec_ns = (_time.perf_counter() - t0) / time_iters * 1e9

    results = [
        {nm: np.asarray(out_arrs[i]).reshape(n_cores, *out_avals[i].shape)[c]
         for i, nm in enumerate(out_names)}
        for c in range(n_cores)
    ]
    return results, exec_ns


def kernel(char_ids, lengths, emb, W_ih_f, W_hh_f, b_ih_f, b_hh_f,
           W_ih_b, W_hh_b, b_ih_b, b_hh_b):
    char_ids = np.asarray(char_ids)
    lengths = np.asarray(lengths)

    # ---- host precompute: fold emb + input proj + biases into G [VOCAB, 4H]
    # permute gate order (i,f,g,o) -> (i,f,o,g) so ACT can evacuate
    # sigmoid-gates [i,f,o] with one instruction
    perm = np.concatenate([np.arange(0, 512),            # i, f
                           np.arange(768, 1024),         # o
                           np.arange(512, 768)])         # g
    sig2 = os.environ.get("K_SIG2", "0") == "1"
    outs = {}
    for d, (W_ih, W_hh, b_ih, b_hh) in enumerate(
            [(W_ih_f, W_hh_f, b_ih_f, b_hh_f),
             (W_ih_b, W_hh_b, b_ih_b, b_hh_b)]):
        G = (np.asarray(emb, np.float64) @ np.asarray(W_ih, np.float64).T
             + np.asarray(b_ih, np.float64) + np.asarray(b_hh, np.float64))
        Gp = np.ascontiguousarray(G[:, perm])
        Wp = np.asarray(W_hh, np.float64)[perm, :].T  # [HID, 4H]
        Wp = np.ascontiguousarray(Wp)
        if sig2:
            # tanh(g) computed as 2*sigmoid(2g)-1: pre-scale g block (cols
            # 768:1024 after permutation) by 2
            Gp[:, 768:1024] *= 2.0
            Wp[:, 768:1024] *= 2.0
        outs[f"G{d}"] = Gp.astype(np.float32)
        outs[f"Wp{d}"] = Wp.astype(np.float32)

    # gt blob: [128, 2, 8, 128]
    gt = np.zeros((128, 2, 8, 128), np.float32)
    for d in range(2):
        for m in range(8):
            gt[:, d, m, :] = outs[f"G{d}"][:, m * 128:(m + 1) * 128]
    # whh blob: [128, 2, 2, 8, 128]
    whh = np.zeros((128, 2, 2, 8, 128), np.float32)
    for d in range(2):
        for k in range(2):
            for m in range(8):
                whh[:, d, k, m, :] = outs[f"Wp{d}"][
                    k * 128:(k + 1) * 128, m * 128:(m + 1) * 128]
    gt = gt.reshape(128, -1)
    whh = whh.reshape(128, -1)

    # ---- schedule
    core_words, col_lens, bins = _build_schedule(lengths)
    emits = [_emit_ranges(col_lens, s, W, S) for (s, W, S) in bins]
    lws = [_live_widths(col_lens, s, W, S) for (s, W, S) in bins]
    TOT = sum(2 * W * S for (_, W, S) in bins)
    OUTCOLS = sum(4 * W for (_, W, _) in bins)

    # ---- one-hot blobs per core
    in_maps = []
    for cidx in range(NCORES):
        words = core_words[cidx]
        oh = np.zeros((128, TOT), np.float32)
        off = 0
        for (start, W, S) in bins:
            w_ids = words[start:start + W]
            lens = col_lens[start:start + W]
            cols = np.arange(W)
            real = w_ids >= 0
            for t in range(S):
                valid = real & (t < lens)
                if valid.any():
                    wv = w_ids[valid]
                    # fwd: char at position t
                    rows_f = char_ids[wv, t]
                    oh[rows_f, off + cols[valid]] = 1.0
                    # bwd: char at position len-1-t
                    rows_b = char_ids[wv, lens[valid] - 1 - t]
                    oh[rows_b, off + W + cols[valid]] = 1.0
                off += 2 * W
        in_maps.append({"oneh": oh, "whh": whh, "gt": gt})

    # ---- build + run
    nc = _build_bass(bins, emits, TOT, OUTCOLS, lws=lws)
    iters = int(os.environ.get("KERNEL_TIME_ITERS", "0"))
    results, exec_ns = _run_spmd(nc, in_maps, time_iters=iters)
    _LAST_RESULT.clear()
    _LAST_RESULT["exec_time_ns"] = exec_ns
    _LAST_RESULT["nc"] = nc

    # ---- assemble output
    final = np.zeros((N_WORDS, 2 * HID), np.float32)
    for cidx in range(NCORES):
        out = results[cidx]["out"]
        words = core_words[cidx]
        ob = 0
        for (start, W, S) in bins:
            w_ids = words[start:start + W]
            real = w_ids >= 0
            for d in range(2):
                block = out[:, ob + d * 2 * W: ob + (d + 1) * 2 * W]
                hv = block.reshape(128, 2, W).transpose(2, 1, 0).reshape(W, 256)
                final[w_ids[real], d * HID:(d + 1) * HID] = hv[real]
            ob += 4 * W
    return final


# revision 34
# speedup vs baseline: 108.5548x; 1.0098x over previous
"""CharBiLSTM Trainium2 kernel.

Strategy:
- Embedding lookup + input projection folded into G = emb @ W_ih.T + b (host),
  so per-step input contribution is a one-hot matmul (one-hot built on host).
- Words sorted by length into 256-wide bins (equal structure across the 8
  cores -> one SPMD program); each bin runs only max-len-in-bin steps and
  finished words' hidden states are emitted at their own final step.
- Feature-major LSTM state ([hid, words]) so the recurrence needs zero
  transposes; gates evacuated from PSUM by ScalarE with fused sigmoid/tanh.
- fp32r matmuls (full PE rate at N>=256, ~1e-4 relative error).
"""

import os

import numpy as np

N_WORDS, MAX_LEN = 16384, 16
VOCAB, EMB, HID = 128, 64, 256
NCORES = 8
BIN_W = 256  # words per bin (free dim of all matmuls); PSUM-bank limited

_LAST_RESULT = {}  # test introspection: exec_time_ns etc.


def _build_schedule(lengths):
    """Sort words by length; build per-core column schedule.

    Returns:
      core_words: [NCORES][Q] word ids (-1 = dummy), identical len-structure
      col_lens:   [Q] length of each column (same for every core)
      bins:       list of (start_col, W, S) with W == BIN_W
    """
    lengths = np.asarray(lengths)
    per_core = [[] for _ in range(NCORES)]
    col_lens = []
    # descending length order: the ragged tail bin (dummy-padded) then has
    # S=1 instead of S=16
    for L in range(MAX_LEN, 0, -1):
        idx = np.where(lengths == L)[0]
        q = -(-len(idx) // NCORES)  # ceil
        pad = q * NCORES - len(idx)
        if pad:
            idx = np.concatenate([idx, np.full(pad, -1, np.int64)])
        for c in range(NCORES):
            per_core[c].extend(idx[c * q:(c + 1) * q].tolist())
        col_lens.extend([L] * q)
    Q = len(col_lens)
    nbins = -(-Q // BIN_W)
    tot = nbins * BIN_W
    for c in range(NCORES):
        per_core[c].extend([-1] * (tot - Q))
    # dummy cols get length 0 (never emitted, never one-hot)
    col_lens = col_lens + [0] * (tot - Q)
    col_lens = np.array(col_lens, np.int64)
    bins = []
    for b in range(nbins):
        sl = col_lens[b * BIN_W:(b + 1) * BIN_W]
        S = int(sl.max())
        bins.append((b * BIN_W, BIN_W, S))
    return [np.array(w, np.int64) for w in per_core], col_lens, bins


def _emit_ranges(col_lens, start, W, S):
    """For each step t (0-based), the [a,b) column range with len == t+1."""
    sl = col_lens[start:start + W]
    out = {}
    for t in range(S):
        cols = np.where(sl == t + 1)[0]
        if len(cols):
            a, b = int(cols[0]), int(cols[-1]) + 1
            assert b - a == len(cols), "columns of equal length must be contiguous"
            out[t] = (a, b)
    return out


def _live_widths(col_lens, start, W, S):
    """Live column count at each step (cols with len >= t+1; descending
    order makes them a prefix). Elementwise/ACT ops are sliced to this;
    matmuls stay full width (fp32r needs N>=256)."""
    sl = col_lens[start:start + W]
    return [int(np.sum(sl >= t + 1)) for t in range(S)]


def _build_bass(bins, emits, TOT, OUTCOLS, lws=None):
    import os
    import concourse.bacc as bacc
    import concourse.tile as tile
    from concourse import mybir

    POOL_T1 = os.environ.get("K_POOL_T1", "0") == "1"
    POOL_T2 = os.environ.get("K_POOL_T2", "0") == "1"
    BUFS = int(os.environ.get("K_BUFS", "2"))
    ABUFS = int(os.environ.get("K_ABUFS", str(BUFS)))
    TBUFS = int(os.environ.get("K_TBUFS", str(BUFS)))
    SPLIT_ACT = os.environ.get("K_SPLIT_ACT", "0") == "1"
    SIG2 = os.environ.get("K_SIG2", "0") == "1"

    f32 = mybir.dt.float32
    f32r = mybir.dt.float32r
    Sig = mybir.ActivationFunctionType.Sigmoid
    Tanh = mybir.ActivationFunctionType.Tanh

    nc = bacc.Bacc(None, target_bir_lowering=False)
    d_oneh = nc.dram_tensor("oneh", [128, TOT], f32r, kind="ExternalInput")
    d_whh = nc.dram_tensor("whh", [128, 2 * 2 * 8 * 128], f32r, kind="ExternalInput")
    d_gt = nc.dram_tensor("gt", [128, 2 * 8 * 128], f32r, kind="ExternalInput")
    d_out = nc.dram_tensor("out", [128, OUTCOLS], f32, kind="ExternalOutput")

    whh_v = d_whh[:, :].rearrange("p (d k m c) -> p d k m c", d=2, k=2, m=8)
    gt_v = d_gt[:, :].rearrange("p (d m c) -> p d m c", d=2, m=8)

    with tile.TileContext(nc) as tc:
        with tc.tile_pool(name="wpool", bufs=1) as wpool, \
             tc.tile_pool(name="ohp", bufs=int(os.environ.get("K_OHB", "4"))) as ohp, \
             tc.tile_pool(name="psp", bufs=1, space="PSUM") as psp, \
             tc.tile_pool(name="actp", bufs=ABUFS) as actp, \
             tc.tile_pool(name="stp", bufs=BUFS) as stp, \
             tc.tile_pool(name="tmpp", bufs=TBUFS) as tmpp:

            whh_sb = wpool.tile([128, 2, 2, 8, 128], f32r)
            nc.sync.dma_start(out=whh_sb, in_=whh_v)
            gt_sb = wpool.tile([128, 2, 8, 128], f32r)
            nc.sync.dma_start(out=gt_sb, in_=gt_v)

            # bin start offsets in the oneh / out blobs
            oh_offs, out_offs = [], []
            acc_oh, acc_out = 0, 0
            for (start, W, S) in bins:
                oh_offs.append(acc_oh)
                out_offs.append(acc_out)
                acc_oh += 2 * W * S
                acc_out += 4 * W

            # group bins (similar step counts adjacent) -> 2*G concurrent
            # recurrence chains sharing the two PSUM slot sets
            G = int(os.environ.get("K_GROUP", "2"))
            order = sorted(range(len(bins)), key=lambda b: -bins[b][2])
            pairs = [order[i:i + G] for i in range(0, len(order), G)]

            for pair in pairs:
                state = {bi: ([None, None], [None, None]) for bi in pair}
                maxS = max(bins[bi][2] for bi in pair)
                for t in range(maxS):
                    active = [bi for bi in pair if t < bins[bi][2]]
                    ohs = {}
                    for ci, bi in enumerate(active):
                        _, W, S = bins[bi]
                        oh = ohp.tile([128, 2, W], f32r, tag="oh", name="oh")
                        nc.sync.dma_start(
                            out=oh,
                            in_=d_oneh[:, oh_offs[bi] + t * 2 * W:
                                       oh_offs[bi] + (t + 1) * 2 * W].rearrange(
                                "p (d w) -> p d w", d=2))
                        ohs[bi] = oh
                    for ci, bi in enumerate(active):
                        if SIG2:
                            _emit_step_sig2(nc, bins[bi], emits[bi], t, ci,
                                            state[bi], ohs[bi], whh_sb, gt_sb,
                                            d_out, out_offs[bi], actp, stp,
                                            tmpp, psp, f32, f32r, Sig, Tanh,
                                            POOL_T1, mybir)
                        else:
                            lw = lws[bi][t] if lws is not None else bins[bi][1]
                            _emit_step(nc, tc, bins[bi], emits[bi], t, ci,
                                       state[bi], ohs[bi], whh_sb, gt_sb, d_out,
                                       out_offs[bi], actp, stp, tmpp, psp,
                                       f32, f32r, Sig, Tanh,
                                       POOL_T1, POOL_T2, SPLIT_ACT, lw)
    nc.compile()
    return nc


def _emit_step(nc, tc, bin_, er, t, ci, state, oh, whh_sb, gt_sb, d_out,
               out_off, actp, stp, tmpp, psp, f32, f32r, Sig, Tanh,
               POOL_T1, POOL_T2, SPLIT_ACT, lw=None):
    start, W, S = bin_
    if lw is None or lw > W:
        lw = W
    h, c = state

    def emit_mms(d, ps_s, ps_g):
        # order: i,f m-tiles (0-3), then g (6,7), then o (4,5)
        # so the i/f sigmoid evacuation can start earliest
        mord = (0, 1, 2, 3, 6, 7, 4, 5) if os.environ.get("K_MORD") == "ifgo" \
            else (0, 1, 2, 3, 4, 5, 6, 7)
        for m in mord:
            o_ap = ps_s[:, m, :] if m < 6 else ps_g[:, m - 6, :]
            nc.tensor.matmul(o_ap, gt_sb[:, d, m, :], oh[:, d, :],
                             start=True, stop=(t == 0))
            if t > 0:
                nc.tensor.matmul(o_ap, whh_sb[:, d, 0, m, :], h[d][:, 0, :],
                                 start=False, stop=False)
                nc.tensor.matmul(o_ap, whh_sb[:, d, 1, m, :], h[d][:, 1, :],
                                 start=False, stop=True)

    JTC = os.environ.get("K_JTC", "0") == "1"

    def emit_gates(d, ps_s, ps_g, c_dst):
        """ACT evac + DVE ops up to writing c_new into c_dst; returns sig."""
        sig = actp.tile([128, 6, W], f32, tag=f"sig{d}{ci}", name=f"sig{d}{ci}")
        tg = actp.tile([128, 2, W], f32, tag=f"tg{d}{ci}", name=f"tg{d}{ci}")
        if SPLIT_ACT:
            nc.scalar.activation(sig[:, 0:4, 0:lw], ps_s[:, 0:4, 0:lw], Sig)
            nc.scalar.activation(tg[:, :, 0:lw], ps_g[:, :, 0:lw], Tanh)
            nc.scalar.activation(sig[:, 4:6, 0:lw], ps_s[:, 4:6, 0:lw], Sig)
        else:
            nc.scalar.activation(sig[:, :, 0:lw], ps_s[:, :, 0:lw], Sig)
            nc.scalar.activation(tg[:, :, 0:lw], ps_g[:, :, 0:lw], Tanh)
        if t == 0:
            nc.vector.tensor_mul(c_dst[:, :, 0:lw], sig[:, 0:2, 0:lw],
                                 tg[:, :, 0:lw])
        else:
            t1 = tmpp.tile([128, 2, W], f32, tag=f"t1{d}{ci}", name=f"t1{d}{ci}")
            eng1 = nc.gpsimd if POOL_T1 else nc.vector
            eng1.tensor_mul(t1[:, :, 0:lw], sig[:, 2:4, 0:lw],
                            c_src(d)[:, :, 0:lw])
            t2 = tmpp.tile([128, 2, W], f32, tag=f"t2{d}{ci}", name=f"t2{d}{ci}")
            eng2 = nc.gpsimd if POOL_T2 else nc.vector
            eng2.tensor_mul(t2[:, :, 0:lw], sig[:, 0:2, 0:lw], tg[:, :, 0:lw])
            nc.vector.tensor_add(c_dst[:, :, 0:lw], t1[:, :, 0:lw],
                                 t2[:, :, 0:lw])
        return sig

    def finish_h(d, sig, tc_ap):
        h_new = stp.tile([128, 2, W], f32r, tag=f"h{d}{ci}", name=f"h{d}{ci}")
        nc.vector.tensor_mul(h_new[:, :, 0:lw], sig[:, 4:6, 0:lw], tc_ap)
        h[d] = h_new
        if t in er:
            a, b = er[t]
            dst = d_out[:, out_off + d * 2 * W:
                        out_off + (d + 1) * 2 * W].rearrange(
                "p (hh w) -> p hh w", hh=2)[:, :, a:b]
            nc.sync.dma_start(out=dst, in_=h_new[:, :, a:b].bitcast(f32))

    ps = {}
    for d in (0, 1):
        ps[d] = (psp.tile([128, 6, W], f32, tag=f"pss{d}", name=f"pss{d}"),
                 psp.tile([128, 2, W], f32, tag=f"psg{d}", name=f"psg{d}"))

    if JTC:
        # both dirs share one c tile; tanh(c) runs as ONE ACT instruction
        cj = stp.tile([128, 2, 2, W], f32, tag=f"cj{ci}", name=f"cj{ci}")
        c_src = lambda d: c[0][:, d]
        sigs = {}
        for d in (0, 1):
            emit_mms(d, *ps[d])
            sigs[d] = emit_gates(d, *ps[d], cj[:, d])
        tcj = tmpp.tile([128, 2, 2, W], f32, tag=f"tcj{ci}", name=f"tcj{ci}")
        nc.scalar.activation(tcj[:, :, :, 0:lw], cj[:, :, :, 0:lw], Tanh)
        for d in (0, 1):
            finish_h(d, sigs[d], tcj[:, d, :, 0:lw])
        c[0] = cj
    else:
        c_src = lambda d: c[d]
        for d in (0, 1):
            emit_mms(d, *ps[d])
            c_new = stp.tile([128, 2, W], f32, tag=f"c{d}{ci}", name=f"c{d}{ci}")
            sig = emit_gates(d, *ps[d], c_new)
            tc_t = tmpp.tile([128, 2, W], f32, tag=f"tc{d}{ci}",
                             name=f"tc{d}{ci}")
            nc.scalar.activation(tc_t[:, :, 0:lw], c_new[:, :, 0:lw], Tanh)
            finish_h(d, sig, tc_t[:, :, 0:lw])
            c[d] = c_new


def _emit_step_sig2(nc, bin_, er, t, ci, state, oh, whh_sb, gt_sb, d_out,
                    out_off, actp, stp, tmpp, psp, f32, f32r, Sig, Tanh,
                    POOL_T1, mybir):
    """Variant: g-gate pre-scaled by 2 on host so tanh(g) = 2*sigmoid(2g)-1;
    all four gates evacuate through ONE sigmoid ACT instruction."""
    start, W, S = bin_
    h, c = state
    mult = mybir.AluOpType.mult

    for d in (0, 1):
        ps = psp.tile([128, 8, W], f32, tag=f"ps{d}", name=f"ps{d}")
        mord = (0, 1, 2, 3, 6, 7, 4, 5) if os.environ.get("K_MORD") == "ifgo" \
            else (0, 1, 2, 3, 4, 5, 6, 7)
        for m in mord:
            o_ap = ps[:, m, :]
            nc.tensor.matmul(o_ap, gt_sb[:, d, m, :], oh[:, d, :],
                             start=True, stop=(t == 0))
            if t > 0:
                nc.tensor.matmul(o_ap, whh_sb[:, d, 0, m, :], h[d][:, 0, :],
                                 start=False, stop=False)
                nc.tensor.matmul(o_ap, whh_sb[:, d, 1, m, :], h[d][:, 1, :],
                                 start=False, stop=True)
        sg = actp.tile([128, 8, W], f32, tag=f"sg{d}{ci}", name=f"sg{d}{ci}")
        nc.scalar.activation(sg, ps, Sig)
        si, sf, so, gg = (sg[:, 0:2, :], sg[:, 2:4, :],
                          sg[:, 4:6, :], sg[:, 6:8, :])
        c_new = stp.tile([128, 2, W], f32, tag=f"c{d}{ci}", name=f"c{d}{ci}")
        t2 = tmpp.tile([128, 2, W], f32, tag=f"t2{d}{ci}", name=f"t2{d}{ci}")
        # t2 = (2*sigmoid(2g)) * sigmoid(i)
        nc.vector.scalar_tensor_tensor(t2, gg, 2.0, si, op0=mult, op1=mult)
        if t == 0:
            nc.vector.tensor_sub(c_new, t2, si)  # c = i*(2sg-1)
        else:
            t1 = tmpp.tile([128, 2, W], f32, tag=f"t1{d}{ci}", name=f"t1{d}{ci}")
            eng1 = nc.gpsimd if POOL_T1 else nc.vector
            eng1.tensor_mul(t1, sf, c[d])
            t3 = tmpp.tile([128, 2, W], f32, tag=f"t3{d}{ci}", name=f"t3{d}{ci}")
            nc.vector.tensor_sub(t3, t1, si)
            nc.vector.tensor_add(c_new, t3, t2)
        tc_t = tmpp.tile([128, 2, W], f32, tag=f"tc{d}{ci}", name=f"tc{d}{ci}")
        nc.scalar.activation(tc_t, c_new, Tanh)
        h_new = stp.tile([128, 2, W], f32r, tag=f"h{d}{ci}", name=f"h{d}{ci}")
        nc.vector.tensor_mul(h_new, so, tc_t)
        h[d], c[d] = h_new, c_new
        if t in er:
            a, b = er[t]
            dst = d_out[:, out_off + d * 2 * W:
                        out_off + (d + 1) * 2 * W].rearrange(
                "p (hh w) -> p hh w", hh=2)[:, :, a:b]
            nc.sync.dma_start(out=dst, in_=h_new[:, :, a:b].bitcast(f32))


def _make_runner(nc, n_cores):
    """Build a reusable jitted SPMD executor for a compiled Bass module.

    Mirrors concourse.bass2jax.run_bass_via_pjrt's shard_map path, but
    keeps the jitted function so repeat calls (for timing) reuse the
    compiled NEFF instead of recompiling.
    """
    import jax
    from jax.sharding import Mesh, PartitionSpec
    from jax.experimental.shard_map import shard_map
    from concourse import bass2jax, mybir

    bass2jax.install_neuronx_cc_hook()
    assert nc.dbg_addr is None
    part_name = nc.partition_id_tensor.name if nc.partition_id_tensor else None

    in_names, out_names, out_avals, zero_outs = [], [], [], []
    for alloc in nc.m.functions[0].allocations:
        if not isinstance(alloc, mybir.MemoryLocationSet):
            continue
        name = alloc.memorylocations[0].name
        if alloc.kind == "ExternalInput":
            if name != part_name:
                in_names.append(name)
        elif alloc.kind == "ExternalOutput":
            np_dt = mybir.dt.np(alloc.dtype)
            shape = tuple(alloc.tensor_shape)
            out_avals.append(jax.core.ShapedArray(shape, np_dt))
            out_names.append(name)
            zero_outs.append(np.zeros(shape, np_dt))
    n_params = len(in_names)
    all_names = in_names + out_names
    if part_name is not None:
        all_names = all_names + [part_name]

    def _body(*args):
        operands = list(args)
        if part_name is not None:
            operands.append(bass2jax.partition_id_tensor())
        outs = bass2jax._bass_exec_p.bind(
            *operands,
            out_avals=tuple(out_avals),
            in_names=tuple(all_names),
            out_names=tuple(out_names),
            lowering_input_output_aliases=(),
            sim_require_finite=True,
            sim_require_nnan=True,
            nc=nc,
        )
        return tuple(outs)

    devices = jax.devices()[:n_cores]
    mesh = Mesh(np.asarray(devices), ("core",))
    nin = n_params + len(zero_outs)
    sharded = jax.jit(
        shard_map(_body, mesh=mesh,
                  in_specs=(PartitionSpec("core"),) * nin,
                  out_specs=(PartitionSpec("core"),) * len(out_names),
                  check_rep=False),
        keep_unused=True,
    )
    return sharded, in_names, out_names, out_avals, zero_outs


def _run_spmd(nc, in_maps, time_iters=0):
    """Execute once (returns per-core result dicts); optionally time."""
    import time as _time
    import jax

    n_cores = len(in_maps)
    sharded, in_names, out_names, out_avals, zero_outs = _make_runner(nc, n_cores)
    concat_in = [
        np.concatenate([np.asarray(in_maps[c][nm]) for c in range(n_cores)], axis=0)
        for nm in in_names
    ]
    concat_zeros = [
        np.zeros((n_cores * z.shape[0], *z.shape[1:]), z.dtype) for z in zero_outs
    ]
    dev_args = [jax.device_put(a) for a in concat_in + concat_zeros]
    out_arrs = sharded(*dev_args)
    jax.block_until_ready(out_arrs)

    exec_ns = None
    if time_iters:
        # warm
        jax.block_until_ready(sharded(*dev_args))
        t0 = _time.perf_counter()
        last = None
        for _ in range(time_iters):
            last = sharded(*dev_args)
        jax.block_until_ready(last)
        exec_ns = (_time.perf_counter() - t0) / time_iters * 1e9

    results = [
        {nm: np.asarray(out_arrs[i]).reshape(n_cores, *out_avals[i].shape)[c]
         for i, nm in enumerate(out_names)}
        for c in range(n_cores)
    ]
    return results, exec_ns


def kernel(char_ids, lengths, emb, W_ih_f, W_hh_f, b_ih_f, b_hh_f,
           W_ih_b, W_hh_b, b_ih_b, b_hh_b):
    char_ids = np.asarray(char_ids)
    lengths = np.asarray(lengths)

    # ---- host precompute: fold emb + input proj + biases into G [VOCAB, 4H]
    # permute gate order (i,f,g,o) -> (i,f,o,g) so ACT can evacuate
    # sigmoid-gates [i,f,o] with one instruction
    perm = np.concatenate([np.arange(0, 512),            # i, f
                           np.arange(768, 1024),         # o
                           np.arange(512, 768)])         # g
    sig2 = os.environ.get("K_SIG2", "0") == "1"
    outs = {}
    for d, (W_ih, W_hh, b_ih, b_hh) in enumerate(
            [(W_ih_f, W_hh_f, b_ih_f, b_hh_f),
             (W_ih_b, W_hh_b, b_ih_b, b_hh_b)]):
        G = (np.asarray(emb, np.float64) @ np.asarray(W_ih, np.float64).T
             + np.asarray(b_ih, np.float64) + np.asarray(b_hh, np.float64))
        Gp = np.ascontiguousarray(G[:, perm])
        Wp = np.asarray(W_hh, np.float64)[perm, :].T  # [HID, 4H]
        Wp = np.ascontiguousarray(Wp)
        if sig2:
            # tanh(g) computed as 2*sigmoid(2g)-1: pre-scale g block (cols
            # 768:1024 after permutation) by 2
            Gp[:, 768:1024] *= 2.0
            Wp[:, 768:1024] *= 2.0
        outs[f"G{d}"] = Gp.astype(np.float32)
        outs[f"Wp{d}"] = Wp.astype(np.float32)

    # gt blob: [128, 2, 8, 128]
    gt = np.zeros((128, 2, 8, 128), np.float32)
    for d in range(2):
        for m in range(8):
            gt[:, d, m, :] = outs[f"G{d}"][:, m * 128:(m + 1) * 128]
    # whh blob: [128, 2, 2, 8, 128]
    whh = np.zeros((128, 2, 2, 8, 128), np.float32)
    for d in range(2):
        for k in range(2):
            for m in range(8):
                whh[:, d, k, m, :] = outs[f"Wp{d}"][
                    k * 128:(k + 1) * 128, m * 128:(m + 1) * 128]
    gt = gt.reshape(128, -1)
    whh = whh.reshape(128, -1)

    # ---- schedule
    core_words, col_lens, bins = _build_schedule(lengths)
    emits = [_emit_ranges(col_lens, s, W, S) for (s, W, S) in bins]
    lws = [_live_widths(col_lens, s, W, S) for (s, W, S) in bins]
    TOT = sum(2 * W * S for (_, W, S) in bins)
    OUTCOLS = sum(4 * W for (_, W, _) in bins)

    # ---- one-hot blobs per core
    in_maps = []
    for cidx in range(NCORES):
        words = core_words[cidx]
        oh = np.zeros((128, TOT), np.float32)
        off = 0
        for (start, W, S) in bins:
            w_ids = words[start:start + W]
            lens = col_lens[start:start + W]
            cols = np.arange(W)
            real = w_ids >= 0
            for t in range(S):
                valid = real & (t < lens)
                if valid.any():
                    wv = w_ids[valid]
                    # fwd: char at position t
                    rows_f = char_ids[wv, t]
                    oh[rows_f, off + cols[valid]] = 1.0
                    # bwd: char at position len-1-t
                    rows_b = char_ids[wv, lens[valid] - 1 - t]
                    oh[rows_b, off + W + cols[valid]] = 1.0
                off += 2 * W
        in_maps.append({"oneh": oh, "whh": whh, "gt": gt})

    # ---- build + run
    nc = _build_bass(bins, emits, TOT, OUTCOLS, lws=lws)
    iters = int(os.environ.get("KERNEL_TIME_ITERS", "0"))
    results, exec_ns = _run_spmd(nc, in_maps, time_iters=iters)
    _LAST_RESULT.clear()
    _LAST_RESULT["exec_time_ns"] = exec_ns
    _LAST_RESULT["nc"] = nc

    # ---- assemble output
    final = np.zeros((N_WORDS, 2 * HID), np.float32)
    for cidx in range(NCORES):
        out = results[cidx]["out"]
        words = core_words[cidx]
        ob = 0
        for (start, W, S) in bins:
            w_ids = words[start:start + W]
            real = w_ids >= 0
            for d in range(2):
                block = out[:, ob + d * 2 * W: ob + (d + 1) * 2 * W]
                hv = block.reshape(128, 2, W).transpose(2, 1, 0).reshape(W, 256)
                final[w_ids[real], d * HID:(d + 1) * HID] = hv[real]
            ob += 4 * W
    return final


# revision 35
# speedup vs baseline: 110.0035x; 1.0133x over previous
"""CharBiLSTM Trainium2 kernel.

Strategy:
- Embedding lookup + input projection folded into G = emb @ W_ih.T + b (host),
  so per-step input contribution is a one-hot matmul (one-hot built on host).
- Words sorted by length into 256-wide bins (equal structure across the 8
  cores -> one SPMD program); each bin runs only max-len-in-bin steps and
  finished words' hidden states are emitted at their own final step.
- Feature-major LSTM state ([hid, words]) so the recurrence needs zero
  transposes; gates evacuated from PSUM by ScalarE with fused sigmoid/tanh.
- fp32r matmuls (full PE rate at N>=256, ~1e-4 relative error).
"""

import os

import numpy as np

N_WORDS, MAX_LEN = 16384, 16
VOCAB, EMB, HID = 128, 64, 256
NCORES = 8
BIN_W = 256  # words per bin (free dim of all matmuls); PSUM-bank limited

_LAST_RESULT = {}  # test introspection: exec_time_ns etc.


def _build_schedule(lengths):
    """Sort words by length; build per-core column schedule.

    Returns:
      core_words: [NCORES][Q] word ids (-1 = dummy), identical len-structure
      col_lens:   [Q] length of each column (same for every core)
      bins:       list of (start_col, W, S) with W == BIN_W
    """
    lengths = np.asarray(lengths)
    per_core = [[] for _ in range(NCORES)]
    col_lens = []
    # descending length order: the ragged tail bin (dummy-padded) then has
    # S=1 instead of S=16
    for L in range(MAX_LEN, 0, -1):
        idx = np.where(lengths == L)[0]
        q = -(-len(idx) // NCORES)  # ceil
        pad = q * NCORES - len(idx)
        if pad:
            idx = np.concatenate([idx, np.full(pad, -1, np.int64)])
        for c in range(NCORES):
            per_core[c].extend(idx[c * q:(c + 1) * q].tolist())
        col_lens.extend([L] * q)
    Q = len(col_lens)
    nbins = -(-Q // BIN_W)
    tot = nbins * BIN_W
    for c in range(NCORES):
        per_core[c].extend([-1] * (tot - Q))
    # dummy cols get length 0 (never emitted, never one-hot)
    col_lens = col_lens + [0] * (tot - Q)
    col_lens = np.array(col_lens, np.int64)
    bins = []
    for b in range(nbins):
        sl = col_lens[b * BIN_W:(b + 1) * BIN_W]
        S = int(sl.max())
        bins.append((b * BIN_W, BIN_W, S))
    return [np.array(w, np.int64) for w in per_core], col_lens, bins


def _emit_ranges(col_lens, start, W, S):
    """For each step t (0-based), the [a,b) column range with len == t+1."""
    sl = col_lens[start:start + W]
    out = {}
    for t in range(S):
        cols = np.where(sl == t + 1)[0]
        if len(cols):
            a, b = int(cols[0]), int(cols[-1]) + 1
            assert b - a == len(cols), "columns of equal length must be contiguous"
            out[t] = (a, b)
    return out


def _live_widths(col_lens, start, W, S):
    """Live column count at each step (cols with len >= t+1; descending
    order makes them a prefix). Elementwise/ACT ops are sliced to this;
    matmuls stay full width (fp32r needs N>=256)."""
    sl = col_lens[start:start + W]
    return [int(np.sum(sl >= t + 1)) for t in range(S)]


def _build_bass(bins, emits, TOT, OUTCOLS, lws=None):
    import os
    import concourse.bacc as bacc
    import concourse.tile as tile
    from concourse import mybir

    POOL_T1 = os.environ.get("K_POOL_T1", "0") == "1"
    POOL_T2 = os.environ.get("K_POOL_T2", "0") == "1"
    BUFS = int(os.environ.get("K_BUFS", "2"))
    ABUFS = int(os.environ.get("K_ABUFS", str(BUFS)))
    TBUFS = int(os.environ.get("K_TBUFS", str(BUFS)))
    SPLIT_ACT = os.environ.get("K_SPLIT_ACT", "0") == "1"
    SIG2 = os.environ.get("K_SIG2", "0") == "1"

    f32 = mybir.dt.float32
    f32r = mybir.dt.float32r
    Sig = mybir.ActivationFunctionType.Sigmoid
    Tanh = mybir.ActivationFunctionType.Tanh

    nc = bacc.Bacc(None, target_bir_lowering=False)
    d_oneh = nc.dram_tensor("oneh", [128, TOT], f32r, kind="ExternalInput")
    d_whh = nc.dram_tensor("whh", [128, 2 * 2 * 8 * 128], f32r, kind="ExternalInput")
    d_gt = nc.dram_tensor("gt", [128, 2 * 8 * 128], f32r, kind="ExternalInput")
    d_out = nc.dram_tensor("out", [128, OUTCOLS], f32, kind="ExternalOutput")

    whh_v = d_whh[:, :].rearrange("p (d k m c) -> p d k m c", d=2, k=2, m=8)
    gt_v = d_gt[:, :].rearrange("p (d m c) -> p d m c", d=2, m=8)

    with tile.TileContext(nc) as tc:
        with tc.tile_pool(name="wpool", bufs=1) as wpool, \
             tc.tile_pool(name="ohp", bufs=int(os.environ.get("K_OHB", "4"))) as ohp, \
             tc.tile_pool(name="psp", bufs=1, space="PSUM") as psp, \
             tc.tile_pool(name="actp", bufs=ABUFS) as actp, \
             tc.tile_pool(name="stp", bufs=BUFS) as stp, \
             tc.tile_pool(name="tmpp", bufs=TBUFS) as tmpp:

            whh_sb = wpool.tile([128, 2, 2, 8, 128], f32r)
            nc.sync.dma_start(out=whh_sb, in_=whh_v)
            gt_sb = wpool.tile([128, 2, 8, 128], f32r)
            nc.sync.dma_start(out=gt_sb, in_=gt_v)

            # bin start offsets in the oneh / out blobs
            oh_offs, out_offs = [], []
            acc_oh, acc_out = 0, 0
            for (start, W, S) in bins:
                oh_offs.append(acc_oh)
                out_offs.append(acc_out)
                acc_oh += 2 * W * S
                acc_out += 4 * W

            # group bins (similar step counts adjacent) -> 2*G concurrent
            # recurrence chains sharing the two PSUM slot sets
            G = int(os.environ.get("K_GROUP", "2"))
            order = sorted(range(len(bins)), key=lambda b: -bins[b][2])
            pairs = [order[i:i + G] for i in range(0, len(order), G)]

            for pair in pairs:
                state = {bi: ([None, None], [None, None]) for bi in pair}
                maxS = max(bins[bi][2] for bi in pair)
                for t in range(maxS):
                    active = [bi for bi in pair if t < bins[bi][2]]
                    ohs = {}
                    for ci, bi in enumerate(active):
                        _, W, S = bins[bi]
                        oh = ohp.tile([128, 2, W], f32r, tag="oh", name="oh")
                        nc.sync.dma_start(
                            out=oh,
                            in_=d_oneh[:, oh_offs[bi] + t * 2 * W:
                                       oh_offs[bi] + (t + 1) * 2 * W].rearrange(
                                "p (d w) -> p d w", d=2))
                        ohs[bi] = oh
                    for ci, bi in enumerate(active):
                        if SIG2:
                            _emit_step_sig2(nc, bins[bi], emits[bi], t, ci,
                                            state[bi], ohs[bi], whh_sb, gt_sb,
                                            d_out, out_offs[bi], actp, stp,
                                            tmpp, psp, f32, f32r, Sig, Tanh,
                                            POOL_T1, mybir)
                        else:
                            lw = lws[bi][t] if lws is not None else bins[bi][1]
                            _emit_step(nc, tc, bins[bi], emits[bi], t, ci,
                                       state[bi], ohs[bi], whh_sb, gt_sb, d_out,
                                       out_offs[bi], actp, stp, tmpp, psp,
                                       f32, f32r, Sig, Tanh,
                                       POOL_T1, POOL_T2, SPLIT_ACT, lw)
    nc.compile()
    return nc


def _emit_step(nc, tc, bin_, er, t, ci, state, oh, whh_sb, gt_sb, d_out,
               out_off, actp, stp, tmpp, psp, f32, f32r, Sig, Tanh,
               POOL_T1, POOL_T2, SPLIT_ACT, lw=None):
    start, W, S = bin_
    if lw is None or lw > W:
        lw = W
    h, c = state

    def emit_mms(d, ps_s, ps_g):
        # order: i,f m-tiles (0-3), then g (6,7), then o (4,5)
        # so the i/f sigmoid evacuation can start earliest
        mord = (0, 1, 2, 3, 6, 7, 4, 5) if os.environ.get("K_MORD") == "ifgo" \
            else (0, 1, 2, 3, 4, 5, 6, 7)
        for m in mord:
            if t == 0 and m in (2, 3):
                continue  # forget gate unused at t=0 (c=0)
            o_ap = ps_s[:, m, :] if m < 6 else ps_g[:, m - 6, :]
            nc.tensor.matmul(o_ap, gt_sb[:, d, m, :], oh[:, d, :],
                             start=True, stop=(t == 0))
            if t > 0:
                nc.tensor.matmul(o_ap, whh_sb[:, d, 0, m, :], h[d][:, 0, :],
                                 start=False, stop=False)
                nc.tensor.matmul(o_ap, whh_sb[:, d, 1, m, :], h[d][:, 1, :],
                                 start=False, stop=True)

    JTC = os.environ.get("K_JTC", "0") == "1"

    def emit_gates(d, ps_s, ps_g, c_dst):
        """ACT evac + DVE ops up to writing c_new into c_dst; returns sig."""
        sig = actp.tile([128, 6, W], f32, tag=f"sig{d}{ci}", name=f"sig{d}{ci}")
        tg = actp.tile([128, 2, W], f32, tag=f"tg{d}{ci}", name=f"tg{d}{ci}")
        if SPLIT_ACT:
            nc.scalar.activation(sig[:, 0:4, 0:lw], ps_s[:, 0:4, 0:lw], Sig)
            nc.scalar.activation(tg[:, :, 0:lw], ps_g[:, :, 0:lw], Tanh)
            nc.scalar.activation(sig[:, 4:6, 0:lw], ps_s[:, 4:6, 0:lw], Sig)
        elif t == 0:
            nc.scalar.activation(sig[:, 0:2, 0:lw], ps_s[:, 0:2, 0:lw], Sig)
            nc.scalar.activation(sig[:, 4:6, 0:lw], ps_s[:, 4:6, 0:lw], Sig)
            nc.scalar.activation(tg[:, :, 0:lw], ps_g[:, :, 0:lw], Tanh)
        else:
            nc.scalar.activation(sig[:, :, 0:lw], ps_s[:, :, 0:lw], Sig)
            nc.scalar.activation(tg[:, :, 0:lw], ps_g[:, :, 0:lw], Tanh)
        if t == 0:
            nc.vector.tensor_mul(c_dst[:, :, 0:lw], sig[:, 0:2, 0:lw],
                                 tg[:, :, 0:lw])
        else:
            t1 = tmpp.tile([128, 2, W], f32, tag=f"t1{d}{ci}", name=f"t1{d}{ci}")
            eng1 = nc.gpsimd if POOL_T1 else nc.vector
            eng1.tensor_mul(t1[:, :, 0:lw], sig[:, 2:4, 0:lw],
                            c_src(d)[:, :, 0:lw])
            t2 = tmpp.tile([128, 2, W], f32, tag=f"t2{d}{ci}", name=f"t2{d}{ci}")
            eng2 = nc.gpsimd if POOL_T2 else nc.vector
            eng2.tensor_mul(t2[:, :, 0:lw], sig[:, 0:2, 0:lw], tg[:, :, 0:lw])
            nc.vector.tensor_add(c_dst[:, :, 0:lw], t1[:, :, 0:lw],
                                 t2[:, :, 0:lw])
        return sig

    def finish_h(d, sig, tc_ap):
        h_new = stp.tile([128, 2, W], f32r, tag=f"h{d}{ci}", name=f"h{d}{ci}")
        nc.vector.tensor_mul(h_new[:, :, 0:lw], sig[:, 4:6, 0:lw], tc_ap)
        h[d] = h_new
        if t in er:
            a, b = er[t]
            dst = d_out[:, out_off + d * 2 * W:
                        out_off + (d + 1) * 2 * W].rearrange(
                "p (hh w) -> p hh w", hh=2)[:, :, a:b]
            nc.sync.dma_start(out=dst, in_=h_new[:, :, a:b].bitcast(f32))

    ps = {}
    for d in (0, 1):
        ps[d] = (psp.tile([128, 6, W], f32, tag=f"pss{d}", name=f"pss{d}"),
                 psp.tile([128, 2, W], f32, tag=f"psg{d}", name=f"psg{d}"))

    if JTC:
        # both dirs share one c tile; tanh(c) runs as ONE ACT instruction
        cj = stp.tile([128, 2, 2, W], f32, tag=f"cj{ci}", name=f"cj{ci}")
        c_src = lambda d: c[0][:, d]
        sigs = {}
        for d in (0, 1):
            emit_mms(d, *ps[d])
            sigs[d] = emit_gates(d, *ps[d], cj[:, d])
        tcj = tmpp.tile([128, 2, 2, W], f32, tag=f"tcj{ci}", name=f"tcj{ci}")
        nc.scalar.activation(tcj[:, :, :, 0:lw], cj[:, :, :, 0:lw], Tanh)
        for d in (0, 1):
            finish_h(d, sigs[d], tcj[:, d, :, 0:lw])
        c[0] = cj
    else:
        c_src = lambda d: c[d]
        for d in (0, 1):
            emit_mms(d, *ps[d])
            c_new = stp.tile([128, 2, W], f32, tag=f"c{d}{ci}", name=f"c{d}{ci}")
            sig = emit_gates(d, *ps[d], c_new)
            tc_t = tmpp.tile([128, 2, W], f32, tag=f"tc{d}{ci}",
                             name=f"tc{d}{ci}")
            nc.scalar.activation(tc_t[:, :, 0:lw], c_new[:, :, 0:lw], Tanh)
            finish_h(d, sig, tc_t[:, :, 0:lw])
            c[d] = c_new


def _emit_step_sig2(nc, bin_, er, t, ci, state, oh, whh_sb, gt_sb, d_out,
                    out_off, actp, stp, tmpp, psp, f32, f32r, Sig, Tanh,
                    POOL_T1, mybir):
    """Variant: g-gate pre-scaled by 2 on host so tanh(g) = 2*sigmoid(2g)-1;
    all four gates evacuate through ONE sigmoid ACT instruction."""
    start, W, S = bin_
    h, c = state
    mult = mybir.AluOpType.mult

    for d in (0, 1):
        ps = psp.tile([128, 8, W], f32, tag=f"ps{d}", name=f"ps{d}")
        mord = (0, 1, 2, 3, 6, 7, 4, 5) if os.environ.get("K_MORD") == "ifgo" \
            else (0, 1, 2, 3, 4, 5, 6, 7)
        for m in mord:
            o_ap = ps[:, m, :]
            nc.tensor.matmul(o_ap, gt_sb[:, d, m, :], oh[:, d, :],
                             start=True, stop=(t == 0))
            if t > 0:
                nc.tensor.matmul(o_ap, whh_sb[:, d, 0, m, :], h[d][:, 0, :],
                                 start=False, stop=False)
                nc.tensor.matmul(o_ap, whh_sb[:, d, 1, m, :], h[d][:, 1, :],
                                 start=False, stop=True)
        sg = actp.tile([128, 8, W], f32, tag=f"sg{d}{ci}", name=f"sg{d}{ci}")
        nc.scalar.activation(sg, ps, Sig)
        si, sf, so, gg = (sg[:, 0:2, :], sg[:, 2:4, :],
                          sg[:, 4:6, :], sg[:, 6:8, :])
        c_new = stp.tile([128, 2, W], f32, tag=f"c{d}{ci}", name=f"c{d}{ci}")
        t2 = tmpp.tile([128, 2, W], f32, tag=f"t2{d}{ci}", name=f"t2{d}{ci}")
        # t2 = (2*sigmoid(2g)) * sigmoid(i)
        nc.vector.scalar_tensor_tensor(t2, gg, 2.0, si, op0=mult, op1=mult)
        if t == 0:
            nc.vector.tensor_sub(c_new, t2, si)  # c = i*(2sg-1)
        else:
            t1 = tmpp.tile([128, 2, W], f32, tag=f"t1{d}{ci}", name=f"t1{d}{ci}")
            eng1 = nc.gpsimd if POOL_T1 else nc.vector
            eng1.tensor_mul(t1, sf, c[d])
            t3 = tmpp.tile([128, 2, W], f32, tag=f"t3{d}{ci}", name=f"t3{d}{ci}")
            nc.vector.tensor_sub(t3, t1, si)
            nc.vector.tensor_add(c_new, t3, t2)
        tc_t = tmpp.tile([128, 2, W], f32, tag=f"tc{d}{ci}", name=f"tc{d}{ci}")
        nc.scalar.activation(tc_t, c_new, Tanh)
        h_new = stp.tile([128, 2, W], f32r, tag=f"h{d}{ci}", name=f"h{d}{ci}")
        nc.vector.tensor_mul(h_new, so, tc_t)
        h[d], c[d] = h_new, c_new
        if t in er:
            a, b = er[t]
            dst = d_out[:, out_off + d * 2 * W:
                        out_off + (d + 1) * 2 * W].rearrange(
                "p (hh w) -> p hh w", hh=2)[:, :, a:b]
            nc.sync.dma_start(out=dst, in_=h_new[:, :, a:b].bitcast(f32))


def _make_runner(nc, n_cores):
    """Build a reusable jitted SPMD executor for a compiled Bass module.

    Mirrors concourse.bass2jax.run_bass_via_pjrt's shard_map path, but
    keeps the jitted function so repeat calls (for timing) reuse the
    compiled NEFF instead of recompiling.
    """
    import jax
    from jax.sharding import Mesh, PartitionSpec
    from jax.experimental.shard_map import shard_map
    from concourse import bass2jax, mybir

    bass2jax.install_neuronx_cc_hook()
    assert nc.dbg_addr is None
    part_name = nc.partition_id_tensor.name if nc.partition_id_tensor else None

    in_names, out_names, out_avals, zero_outs = [], [], [], []
    for alloc in nc.m.functions[0].allocations:
        if not isinstance(alloc, mybir.MemoryLocationSet):
            continue
        name = alloc.memorylocations[0].name
        if alloc.kind == "ExternalInput":
            if name != part_name:
                in_names.append(name)
        elif alloc.kind == "ExternalOutput":
            np_dt = mybir.dt.np(alloc.dtype)
            shape = tuple(alloc.tensor_shape)
            out_avals.append(jax.core.ShapedArray(shape, np_dt))
            out_names.append(name)
            zero_outs.append(np.zeros(shape, np_dt))
    n_params = len(in_names)
    all_names = in_names + out_names
    if part_name is not None:
        all_names = all_names + [part_name]

    def _body(*args):
        operands = list(args)
        if part_name is not None:
            operands.append(bass2jax.partition_id_tensor())
        outs = bass2jax._bass_exec_p.bind(
            *operands,
            out_avals=tuple(out_avals),
            in_names=tuple(all_names),
            out_names=tuple(out_names),
            lowering_input_output_aliases=(),
            sim_require_finite=True,
            sim_require_nnan=True,
            nc=nc,
        )
        return tuple(outs)

    devices = jax.devices()[:n_cores]
    mesh = Mesh(np.asarray(devices), ("core",))
    nin = n_params + len(zero_outs)
    sharded = jax.jit(
        shard_map(_body, mesh=mesh,
                  in_specs=(PartitionSpec("core"),) * nin,
                  out_specs=(PartitionSpec("core"),) * len(out_names),
                  check_rep=False),
        keep_unused=True,
    )
    return sharded, in_names, out_names, out_avals, zero_outs


def _run_spmd(nc, in_maps, time_iters=0):
    """Execute once (returns per-core result dicts); optionally time."""
    import time as _time
    import jax

    n_cores = len(in_maps)
    sharded, in_names, out_names, out_avals, zero_outs = _make_runner(nc, n_cores)
    concat_in = [
        np.concatenate([np.asarray(in_maps[c][nm]) for c in range(n_cores)], axis=0)
        for nm in in_names
    ]
    concat_zeros = [
        np.zeros((n_cores * z.shape[0], *z.shape[1:]), z.dtype) for z in zero_outs
    ]
    dev_args = [jax.device_put(a) for a in concat_in + concat_zeros]
    out_arrs = sharded(*dev_args)
    jax.block_until_ready(out_arrs)

    exec_ns = None
    if time_iters:
        # warm
        jax.block_until_ready(sharded(*dev_args))
        t0 = _time.perf_counter()
        last = None
        for _ in range(time_iters):
            last = sharded(*dev_args)
        jax.block_until_ready(last)
        exec_ns = (_time.perf_counter() - t0) / time_iters * 1e9

    results = [
        {nm: np.asarray(out_arrs[i]).reshape(n_cores, *out_avals[i].shape)[c]
         for i, nm in enumerate(out_names)}
        for c in range(n_cores)
    ]
    return results, exec_ns


def kernel(char_ids, lengths, emb, W_ih_f, W_hh_f, b_ih_f, b_hh_f,
           W_ih_b, W_hh_b, b_ih_b, b_hh_b):
    char_ids = np.asarray(char_ids)
    lengths = np.asarray(lengths)

    # ---- host precompute: fold emb + input proj + biases into G [VOCAB, 4H]
    # permute gate order (i,f,g,o) -> (i,f,o,g) so ACT can evacuate
    # sigmoid-gates [i,f,o] with one instruction
    perm = np.concatenate([np.arange(0, 512),            # i, f
                           np.arange(768, 1024),         # o
                           np.arange(512, 768)])         # g
    sig2 = os.environ.get("K_SIG2", "0") == "1"
    outs = {}
    for d, (W_ih, W_hh, b_ih, b_hh) in enumerate(
            [(W_ih_f, W_hh_f, b_ih_f, b_hh_f),
             (W_ih_b, W_hh_b, b_ih_b, b_hh_b)]):
        G = (np.asarray(emb, np.float64) @ np.asarray(W_ih, np.float64).T
             + np.asarray(b_ih, np.float64) + np.asarray(b_hh, np.float64))
        Gp = np.ascontiguousarray(G[:, perm])
        Wp = np.asarray(W_hh, np.float64)[perm, :].T  # [HID, 4H]
        Wp = np.ascontiguousarray(Wp)
        if sig2:
            # tanh(g) computed as 2*sigmoid(2g)-1: pre-scale g block (cols
            # 768:1024 after permutation) by 2
            Gp[:, 768:1024] *= 2.0
            Wp[:, 768:1024] *= 2.0
        outs[f"G{d}"] = Gp.astype(np.float32)
        outs[f"Wp{d}"] = Wp.astype(np.float32)

    # gt blob: [128, 2, 8, 128]
    gt = np.zeros((128, 2, 8, 128), np.float32)
    for d in range(2):
        for m in range(8):
            gt[:, d, m, :] = outs[f"G{d}"][:, m * 128:(m + 1) * 128]
    # whh blob: [128, 2, 2, 8, 128]
    whh = np.zeros((128, 2, 2, 8, 128), np.float32)
    for d in range(2):
        for k in range(2):
            for m in range(8):
                whh[:, d, k, m, :] = outs[f"Wp{d}"][
                    k * 128:(k + 1) * 128, m * 128:(m + 1) * 128]
    gt = gt.reshape(128, -1)
    whh = whh.reshape(128, -1)

    # ---- schedule
    core_words, col_lens, bins = _build_schedule(lengths)
    emits = [_emit_ranges(col_lens, s, W, S) for (s, W, S) in bins]
    lws = [_live_widths(col_lens, s, W, S) for (s, W, S) in bins]
    TOT = sum(2 * W * S for (_, W, S) in bins)
    OUTCOLS = sum(4 * W for (_, W, _) in bins)

    # ---- one-hot blobs per core
    in_maps = []
    for cidx in range(NCORES):
        words = core_words[cidx]
        oh = np.zeros((128, TOT), np.float32)
        off = 0
        for (start, W, S) in bins:
            w_ids = words[start:start + W]
            lens = col_lens[start:start + W]
            cols = np.arange(W)
            real = w_ids >= 0
            for t in range(S):
                valid = real & (t < lens)
                if valid.any():
                    wv = w_ids[valid]
                    # fwd: char at position t
                    rows_f = char_ids[wv, t]
                    oh[rows_f, off + cols[valid]] = 1.0
                    # bwd: char at position len-1-t
                    rows_b = char_ids[wv, lens[valid] - 1 - t]
                    oh[rows_b, off + W + cols[valid]] = 1.0
                off += 2 * W
        in_maps.append({"oneh": oh, "whh": whh, "gt": gt})

    # ---- build + run
    nc = _build_bass(bins, emits, TOT, OUTCOLS, lws=lws)
    iters = int(os.environ.get("KERNEL_TIME_ITERS", "0"))
    results, exec_ns = _run_spmd(nc, in_maps, time_iters=iters)
    _LAST_RESULT.clear()
    _LAST_RESULT["exec_time_ns"] = exec_ns
    _LAST_RESULT["nc"] = nc

    # ---- assemble output
    final = np.zeros((N_WORDS, 2 * HID), np.float32)
    for cidx in range(NCORES):
        out = results[cidx]["out"]
        words = core_words[cidx]
        ob = 0
        for (start, W, S) in bins:
            w_ids = words[start:start + W]
            real = w_ids >= 0
            for d in range(2):
                block = out[:, ob + d * 2 * W: ob + (d + 1) * 2 * W]
                hv = block.reshape(128, 2, W).transpose(2, 1, 0).reshape(W, 256)
                final[w_ids[real], d * HID:(d + 1) * HID] = hv[real]
            ob += 4 * W
    return final
